# revision 45
# baseline (speedup 1.0000x reference)
"""Hypergraph conv kernel, v2.

Pipeline (node-sharded, 8 cores):
  phase1: nfw_table[n] = bf16([exp(a_n)*nf_n (128) | exp(a_n) (4) | pad]),
          expl_table[n] = f32 exp(a_n) (for pass B).
  passA:  per edge-run batch: gather nfw rows, ONE reduce -> [U|D] partial,
          cast bf16, scatter-add into U_table[ET,256] (cols 0:132).
  AR:     one bf16 AllReduce of U_table.
  EA:     EAp[e] = bf16((U/D + ef)/D)   (ef projected during passA window)
  passB:  gather EAp rows per incidence, ONE reduce per run group,
          multiply by expl per chunk, scatter-add into y.

SWDGE gathers/scatters run gen_mode=0: descriptor emission on the Pool
Q7 (~3.3ns/row) is the serial spine; transfers overlap it (engine
releases at doorbell; Tile syncs consumers on the DMA sem).
"""
import numpy as np
from dataclasses import dataclass

import concourse.bass as bass
import concourse.mybir as mybir
import concourse.bacc as bacc
import concourse.tile as tile
import bass_rust
from concourse.library_config import mlp as mlp_library
from concourse._compat import get_trn_type, cdiv

F32 = mybir.dt.float32
BF16 = mybir.dt.bfloat16
I16 = mybir.dt.int16
AX = mybir.AxisListType
ALU = mybir.AluOpType
ACTF = mybir.ActivationFunctionType

ROW = 256          # nfw/U table row width (bf16 elems); cols 0:128 nfw, 128:132 expl
UCOL = 132         # useful cols in nfw/U rows


@dataclass
class Cfg:
    N: int = 100000
    E: int = 25000
    D: int = 128
    H: int = 4
    C: int = 32
    NC: int = 8
    TILE_COLS: int = 30
    NSTAGE: int = 14

    @property
    def NSH(self):
        return self.N // self.NC

    @property
    def NT_ROWS(self):
        return cdiv(self.NSH + 1, 128) * 128

    @property
    def ET_ROWS(self):
        return cdiv(self.E + 1, 128) * 128

    @property
    def DUMMY_NODE(self):
        return self.NSH

    @property
    def JUNK_EDGE(self):
        return self.E


def _runs(keys):
    if len(keys) == 0:
        return (np.zeros(0, np.int64),) * 3
    change = np.flatnonzero(np.diff(keys)) + 1
    starts = np.concatenate([[0], change]).astype(np.int64)
    ends = np.concatenate([change, [len(keys)]]).astype(np.int64)
    return starts, ends - starts, keys[starts].astype(np.int64)


@dataclass
class Sched:
    batches: list          # [(k, tile, c0)]
    groups: list           # [(tile, c0, k, B, b0, chunk)]
    ntiles: int
    nchunks: int
    nbatch: int
    chunk_sizes: list


def _mk_schedule(lens_list, cfg: Cfg) -> Sched:
    sorted_lens = [np.sort(np.asarray(l))[::-1] for l in lens_list]
    nbatch_total = max(cdiv(len(l), 128) for l in sorted_lens)
    batches = []
    for b in range(nbatch_total):
        w = 1
        for ls in sorted_lens:
            if b * 128 < len(ls):
                w = max(w, int(ls[b * 128]))
        batches.append(w)
    assert max(batches) <= cfg.TILE_COLS, \
        f"run length {max(batches)} > TILE_COLS"
    placed = []
    t, c = 0, 0
    for k in batches:
        if c + k > cfg.TILE_COLS:
            t += 1
            c = 0
        placed.append((k, t, c))
        c += k
    ntiles = t + 1 if placed else 1
    nbatch = len(placed)
    nchunks = cdiv(nbatch, cfg.NSTAGE)
    chunk_sizes = [min(cfg.NSTAGE, nbatch - i * cfg.NSTAGE) for i in range(nchunks)]
    groups = []
    for bi, (k, t, c0) in enumerate(placed):
        ch = bi // cfg.NSTAGE
        if groups and groups[-1][0] == t and groups[-1][2] == k \
                and groups[-1][5] == ch \
                and groups[-1][1] + groups[-1][2] * groups[-1][3] == c0 \
                and groups[-1][4] + groups[-1][3] == bi:
            t0, c0g, kg, B, b0, chg = groups[-1]
            groups[-1] = (t0, c0g, kg, B + 1, b0, chg)
        else:
            groups.append((t, c0, k, 1, bi, ch))
    return Sched(placed, groups, ntiles, nchunks, nbatch, chunk_sizes)


def _wrap16(flat):
    assert len(flat) % 16 == 0
    b = flat.reshape(-1, 16).T.astype(np.int16)
    return np.tile(b, (8, 1))


SUBMAX = 30


def _subcols(n):
    return [(i, min(SUBMAX, n - i)) for i in range(0, n, SUBMAX)]


def _mk_streams(sched: Sched, starts, lens, gvals, svals, runvals,
                dummy_g, junk_s, dummy_run, cfg: Cfg):
    TC = cfg.TILE_COLS
    g_arr = np.full((sched.ntiles, TC, 128), dummy_g, np.int64)
    s_arr = np.full((sched.nbatch, 128), junk_s, np.int64)
    r_arr = np.full((sched.nbatch, 128), dummy_run, np.int64)
    order = np.argsort(-lens, kind="stable") if len(lens) else np.zeros(0, np.int64)
    for bi, (k, t, c0) in enumerate(sched.batches):
        idxs = order[bi * 128:(bi + 1) * 128]
        nr = len(idxs)
        if nr:
            st = starts[idxs]
            kr = lens[idxs]
            assert kr[0] <= k
            for kk in np.unique(kr):
                sel = np.flatnonzero(kr == kk)
                gm = gvals[st[sel][None, :] + np.arange(kk)[:, None]]
                g_arr[t, c0:c0 + kk, sel] = gm.T
            s_arr[bi, :nr] = svals[idxs]
            r_arr[bi, :nr] = runvals[idxs]
    g_idx = np.concatenate(
        [_wrap16(g_arr[t, c0:c0 + cc].reshape(-1))
         for t in range(sched.ntiles) for (c0, cc) in _subcols(TC)], axis=1)
    sc_blocks = []
    off = 0
    for nb in sched.chunk_sizes:
        sc_blocks.append(_wrap16(s_arr[off:off + nb].reshape(-1)))
        off += nb
    sc_idx = np.concatenate(sc_blocks, axis=1)
    r_idx = np.concatenate(
        [_wrap16(r_arr[b0:b0 + bb].reshape(-1))
         for (b0, bb) in _subcols(sched.nbatch)], axis=1)
    return g_idx, sc_idx, r_idx


def build_plan(node_idx, edge_idx, cfg: Cfg):
    """Pass A is split into two edge-halves (split at ET_ROWS//2, a slot
    boundary) so each half's partial-U AllReduce can overlap the other
    half's compute. Empty-batch scatter slots target row 0 of the half
    table: they add exact zeros (dummy gathers hit the all-zero nfw row),
    so no junk row is needed."""
    node_idx = np.asarray(node_idx).astype(np.int64)
    edge_idx = np.asarray(edge_idx).astype(np.int64)
    split = (cfg.ET_ROWS // 128 // 2) * 128
    bounds = [(0, split), (split, cfg.ET_ROWS)]
    percore = []
    for m in range(cfg.NC):
        sel = np.flatnonzero(node_idx // cfg.NSH == m)
        nl = node_idx[sel] - m * cfg.NSH
        eg = edge_idx[sel]
        halves = []
        for (lo, hi) in bounds:
            hs = np.flatnonzero((eg >= lo) & (eg < hi))
            sA, lA, vA = _runs(eg[hs])
            halves.append(dict(nl=nl[hs], sA=sA, lA=lA, vA=vA - lo))
        oB = np.argsort(nl, kind="stable")
        nB = nl[oB]
        eB = eg[oB]
        sB, lB, vB = _runs(nB)
        percore.append(dict(halves=halves, eB=eB, sB=sB, lB=lB, vB=vB))
    schedA = [_mk_schedule([c["halves"][h]["lA"] for c in percore], cfg)
              for h in range(2)]
    schedB = _mk_schedule([c["lB"] for c in percore], cfg)
    streams = []
    for c in percore:
        st = {}
        for h in range(2):
            ch = c["halves"][h]
            gA, scA, _ = _mk_streams(
                schedA[h], ch["sA"], ch["lA"],
                gvals=ch["nl"], svals=ch["vA"], runvals=ch["vA"],
                dummy_g=cfg.DUMMY_NODE, junk_s=0,
                dummy_run=0, cfg=cfg)
            st[f"gA{h}"] = gA
            st[f"scA{h}"] = scA
        gB, scB, rB = _mk_streams(
            schedB, c["sB"], c["lB"],
            gvals=c["eB"], svals=c["vB"], runvals=c["vB"],
            dummy_g=cfg.JUNK_EDGE, junk_s=cfg.DUMMY_NODE,
            dummy_run=cfg.DUMMY_NODE, cfg=cfg)
        st.update(gB=gB, scB=scB, rB=rB)
        streams.append(st)
    return schedA, schedB, streams


def _ap(t_ap, off, dims):
    base = t_ap
    part = base.ap[0]
    return bass_rust.AP(base.tensor, base.offset + off, [part] + dims)


def build_bass(cfg: Cfg, schedA: Sched, schedB: Sched, replica_groups):
    import os
    _stops = ["init", "phase1", "passA", "coll", "ea", "full"]
    _stop = _stops.index(os.environ.get("GNN_STOP", "full"))
    TC, NS = cfg.TILE_COLS, cfg.NSTAGE
    H, C = cfg.H, cfg.C
    D = cfg.D
    NT, ET = cfg.NT_ROWS, cfg.ET_ROWS
    NSH, E = cfg.NSH, cfg.E
    n_a_node = NT // 128
    n_a_edge = ET // 128

    nc = bacc.Bacc(get_trn_type() or "TRN2", target_bir_lowering=False, debug=False,
                   num_swdge_queues=4)
    _qrr = [0]

    def _q():
        q = _qrr[0] % 4
        _qrr[0] += 1
        return q

    # ---- I/O ----
    xT = nc.dram_tensor("xT", [D, NT], F32, kind="ExternalInput")
    haT = nc.dram_tensor("haT", [D, ET], F32, kind="ExternalInput")
    Wn = nc.dram_tensor("Wn", [D, H * C], F32, kind="ExternalInput")
    We = nc.dram_tensor("We", [D, H * C], F32, kind="ExternalInput")
    attn = nc.dram_tensor("attn", [128, H * C], F32, kind="ExternalInput")
    bias_t = nc.dram_tensor("bias_t", [128, 16 * H * C], F32, kind="ExternalInput")
    split = (ET // 128 // 2) * 128
    HBs = [split, ET - split]
    n_a_h = [HBs[0] // 128, HBs[1] // 128]
    gA_i = [nc.dram_tensor(f"gA{h}_i", [128, schedA[h].ntiles * TC * 8], I16,
                           kind="ExternalInput") for h in range(2)]
    scA_i = [nc.dram_tensor(f"scA{h}_i", [128, schedA[h].nbatch * 8], I16,
                            kind="ExternalInput") for h in range(2)]
    gB_i = nc.dram_tensor("gB_i", [128, schedB.ntiles * TC * 8], I16, kind="ExternalInput")
    scB_i = nc.dram_tensor("scB_i", [128, schedB.nbatch * 8], I16, kind="ExternalInput")
    rB_i = nc.dram_tensor("rB_i", [128, schedB.nbatch * 8], I16, kind="ExternalInput")
    y = nc.dram_tensor("y", [NT, H * C], F32, kind="ExternalOutput")
    _dbg = os.environ.get("GNN_DEBUG_OUTS", "0") == "1"
    if _dbg:
        nfw_dbg = nc.dram_tensor("nfw_dbg", [NT, ROW], BF16, kind="ExternalOutput")
        U_dbg = nc.dram_tensor("U_dbg", [ET, ROW], BF16, kind="ExternalOutput")
        Ured_dbg = nc.dram_tensor("Ured_dbg", [ET, ROW], BF16, kind="ExternalOutput")
        EAp_dbg = nc.dram_tensor("EAp_dbg", [ET, 128], BF16, kind="ExternalOutput")

    # ---- internal DRAM ----
    nfw_table = nc.dram_tensor("nfw_table", [NT, ROW], BF16)
    expl_table = nc.dram_tensor("expl_table", [NT, 128], BF16)
    ef_table = nc.dram_tensor("ef_table", [ET, 128], BF16)
    U_half = [nc.dram_tensor(f"U_h{h}", [HBs[h], ROW], BF16) for h in range(2)]
    Ured_half = [nc.dram_tensor(f"Ured_h{h}", [HBs[h], ROW], BF16, addr_space="Shared")
                 for h in range(2)]
    EAp = nc.dram_tensor("EAp", [ET, 128], BF16)

    nfw_v = nfw_table[:].rearrange("(a p) c -> p a c", p=128)
    expl_v = expl_table[:].rearrange("(a p) c -> p a c", p=128)
    ef_v = ef_table[:].rearrange("(a p) c -> p a c", p=128)
    U_v = [U_half[h][:].rearrange("(a p) c -> p a c", p=128) for h in range(2)]
    Ured_v = [Ured_half[h][:].rearrange("(a p) c -> p a c", p=128) for h in range(2)]
    EAp_v = EAp[:].rearrange("(a p) c -> p a c", p=128)
    y_v = y[:].rearrange("(a p) c -> p a c", p=128)

    # SWDGE gathers/scatters run gen_mode=0: the Pool engine emits
    # descriptors (~3.3ns/row) and releases at doorbell; the DMA-completion
    # sem is attached by Tile, so consumer/WAR sync is fully tracked.
    with tile.TileContext(nc) as tc:
        with tc.tile_pool(name="const", bufs=1) as cpool:
            nc.gpsimd.load_library(mlp_library)
            Wn_sb = cpool.tile([D, H * C], F32)
            We_sb = cpool.tile([D, H * C], F32)
            attn_sb = cpool.tile([128, H * C], F32)
            bias_sb = cpool.tile([128, 16 * H * C], F32)
            zbf = cpool.tile([128, 16 * ROW], BF16)
            nc.sync.dma_start(Wn_sb[:], Wn[:])
            nc.sync.dma_start(We_sb[:], We[:])
            nc.sync.dma_start(attn_sb[:], attn[:])
            nc.sync.dma_start(bias_sb[:], bias_t[:])
            nc.gpsimd.memset(zbf[:], 0.0)

            # ---------- init: U halves = 0 (scalar HWDGE queue) ----------
            zv = zbf[:].rearrange("p (a c) -> p a c", c=ROW)
            for h in range(2):
                for a0 in range(0, n_a_h[h], 16):
                    aa = min(16, n_a_h[h] - a0)
                    nc.scalar.dma_start(U_v[h][:, a0:a0 + aa, :], zv[:, :aa, :])

            # ---------- phase 1: node projection -> nfw + expl ----------
            if _stop >= 1:
                with (tc.tile_pool(name="p1", bufs=2) as p1,
                    tc.tile_pool(name="p1s", bufs=2) as p1s,
                    tc.tile_pool(name="ps", bufs=4, space="PSUM") as psp):
                  CHK = 16
                  for a0 in range(0, n_a_node, CHK):
                      aa = min(CHK, n_a_node - a0)
                      xc = p1.tile([D, CHK * 128], F32, tag="xc")
                      nc.sync.dma_start(xc[:, :aa * 128], xT[:, a0 * 128:(a0 + aa) * 128])
                      nfst = p1s.tile([128, CHK, 128], F32, tag="nfst")
                      for i in range(aa):
                          mm = psp.tile([128, 128], F32, tag="mm")
                          nc.tensor.matmul(mm[:], xc[:, i * 128:(i + 1) * 128], Wn_sb[:],
                                           start=True, stop=True)
                          nc.vector.tensor_copy(nfst[:, i, :], mm[:])
                      tmp1 = p1s.tile([128, CHK, 128], F32, tag="tmp1")
                      nc.vector.tensor_tensor(
                          out=tmp1[:, :aa, :].rearrange("p a (h c) -> p a h c", h=H),
                          in0=nfst[:, :aa, :].rearrange("p a (h c) -> p a h c", h=H),
                          in1=attn_sb[:].rearrange("p (h c) -> p h c", h=H)
                              .unsqueeze(1).broadcast_to([128, aa, H, C]),
                          op=ALU.mult)
                      praw = p1s.tile([128, CHK, H], F32, tag="praw")
                      nc.vector.tensor_reduce(
                          out=praw[:, :aa, :],
                          in_=tmp1[:, :aa, :].rearrange("p a (h c) -> p a h c", h=H),
                          axis=AX.X, op=ALU.add)
                      estf = p1s.tile([128, CHK, H], F32, tag="estf")
                      nc.scalar.activation(estf[:, :aa, :], praw[:, :aa, :], ACTF.Exp)
                      est = p1s.tile([128, CHK, 128], BF16, tag="est")
                      nc.gpsimd.memset(est[:], 0.0)
                      nc.vector.tensor_copy(est[:, :aa, 0:H], estf[:, :aa, :])
                      nrow = p1s.tile([128, CHK, ROW], BF16, tag="nrow")
                      nc.gpsimd.memset(nrow[:], 0.0)
                      nc.vector.tensor_tensor(
                          out=nrow[:, :aa, 0:128].rearrange("p a (h c) -> p a h c", h=H),
                          in0=nfst[:, :aa, :].rearrange("p a (h c) -> p a h c", h=H),
                          in1=estf[:, :aa, :].unsqueeze(3).broadcast_to([128, aa, H, C]),
                          op=ALU.mult)
                      nc.vector.tensor_copy(nrow[:, :aa, 128:UCOL], estf[:, :aa, :])
                      nc.sync.dma_start(nfw_v[:, a0:a0 + aa, :], nrow[:, :aa, :])
                      nc.sync.dma_start(expl_v[:, a0:a0 + aa, :], est[:, :aa, :])
                  # zero dummy/pad rows (incl. expl cols -> no D contribution)
                  nc.sync.dma_start(nfw_table[NSH:NT, :], zbf[0:NT - NSH, 0:ROW])

            # ---------- edge projection (PE/DVE-light; before passA so its
            # DVE copies don't queue behind passA reduces; ef ready for EA) --
            if _stop >= 1:
                with (tc.tile_pool(name="pe", bufs=2) as pe,
                    tc.tile_pool(name="pes", bufs=2) as pes,
                    tc.tile_pool(name="ps2", bufs=4, space="PSUM") as psp2):
                  CHK = 16
                  for a0 in range(0, n_a_edge, CHK):
                      aa = min(CHK, n_a_edge - a0)
                      hc = pe.tile([D, CHK * 128], F32, tag="hc")
                      nc.sync.dma_start(hc[:, :aa * 128], haT[:, a0 * 128:(a0 + aa) * 128])
                      efst = pes.tile([128, CHK, 128], BF16, tag="efst")
                      for i in range(aa):
                          mm = psp2.tile([128, 128], F32, tag="mm")
                          nc.tensor.matmul(mm[:], hc[:, i * 128:(i + 1) * 128], We_sb[:],
                                           start=True, stop=True)
                          nc.vector.tensor_copy(efst[:, i, :], mm[:])
                      nc.sync.dma_start(ef_v[:, a0:a0 + aa, :], efst[:, :aa, :])

            # ---------- pass A: two edge-halves, AR per half.
            # Each half gets its own pool scope so the collective is
            # emitted OUTSIDE any pool (inside-pool collectives deadlock
            # Tile's barrier machinery), yet between the halves on the
            # Pool queue so AR(h0) flies during half 1's compute. ----------
            if _stop >= 2:
                  for h in range(2):
                    with (tc.tile_pool(name=f"gA{h}", bufs=4) as gpool,
                        tc.tile_pool(name=f"stA{h}", bufs=4) as spool,
                        tc.tile_pool(name=f"idxA{h}", bufs=1) as ipool):
                      gA_sb = [None, None]
                      scA_sb = [None, None]
                      gA_sb[h] = ipool.tile([128, schedA[h].ntiles * TC * 8], I16,
                                            name=f"gAsb{h}")
                      scA_sb[h] = ipool.tile([128, schedA[h].nbatch * 8], I16,
                                             name=f"scAsb{h}")
                      nc.sync.dma_start(gA_sb[h][:], gA_i[h][:])
                      nc.sync.dma_start(scA_sb[h][:], scA_i[h][:])
                      sched = schedA[h]
                      groups_by_tile = {}
                      for g in sched.groups:
                          groups_by_tile.setdefault(g[0], []).append(g)

                      cur_chunk = [0]
                      stag = {}
                      sc_off = [0]

                      def open_chunk():
                          stag["U"] = spool.tile([128, NS, UCOL], F32,
                                                 tag="ustag", name="ustag")

                      def flush_chunk(h=h, sched=sched):
                          ch = cur_chunk[0]
                          nb = sched.chunk_sizes[ch]
                          # full 512B rows (proven 256B-multiple scatter
                          # geometry); pads add zero, zeroed once per
                          # buffer instance and never dirtied after.
                          ubf = spool.tile([128, NS, ROW], BF16,
                                           tag="ubf", name="ubf")
                          if ch < 4:
                              nc.gpsimd.memset(ubf[:, :, UCOL:], 0.0)
                          nc.vector.tensor_copy(ubf[:, :nb, 0:UCOL],
                                                stag["U"][:, :nb, :])
                          nc.gpsimd.dma_scatter_add(
                              U_half[h][:], ubf[:, :nb, :],
                              scA_sb[h][:, sc_off[0]:sc_off[0] + nb * 8],
                              nb * 128, nb * 128, ROW,
                              single_packet=False, queue_num=_q())
                          sc_off[0] += nb * 8
                          cur_chunk[0] += 1

                      open_chunk()
                      for t in range(sched.ntiles):
                          G = gpool.tile([128, TC, ROW], BF16, tag="G")
                          for (c0s, cc) in _subcols(TC):
                              nc.gpsimd.dma_gather(
                                  G[:, c0s:c0s + cc, :], nfw_table[:],
                                  gA_sb[h][:, t * TC * 8 + c0s * 8:
                                        t * TC * 8 + (c0s + cc) * 8],
                                  cc * 128, cc * 128, ROW,
                                  single_packet=False, queue_num=_q())
                          for (_, c0, k, B, b0, ch) in groups_by_tile.get(t, []):
                              if ch != cur_chunk[0]:
                                  flush_chunk()
                                  open_chunk()
                              bpos = b0 - ch * NS
                              nc.vector.tensor_reduce(
                                  out=stag["U"][:, bpos:bpos + B, :],
                                  in_=_ap(G[:], c0 * ROW,
                                          [[k * ROW, B], [1, UCOL], [ROW, k]]),
                                  axis=AX.X, op=ALU.add)
                      flush_chunk()
                    if _stop >= 3:
                        # emitted after this half's pool scope closes, before
                        # the next half's — AR(h) overlaps half h+1 compute
                        nc.gpsimd.collective_compute(
                            "AllReduce", ALU.add,
                            replica_groups=replica_groups,
                            ins=[U_half[h][:]], outs=[Ured_half[h][:]])

            # ---------- y = bias init (needed only by pass B) ----------
            bv = bias_sb[:].rearrange("p (a c) -> p a c", c=H * C)
            for a0 in range(0, n_a_node, 16):
                aa = min(16, n_a_node - a0)
                nc.scalar.dma_start(y_v[:, a0:a0 + aa, :], bv[:, :aa, :])

            # ---------- pass B prologue: index/expl loads (overlap AR) ----------
            if _stop >= 5:
                pb_stack = [tc.tile_pool(name="gB", bufs=6),
                            tc.tile_pool(name="stB", bufs=4),
                            tc.tile_pool(name="idxB", bufs=1)]
                gpoolB, spoolB, ipoolB = [p.__enter__() for p in pb_stack]
                gB_sb = ipoolB.tile([128, schedB.ntiles * TC * 8], I16)
                scB_sb = ipoolB.tile([128, schedB.nbatch * 8], I16)
                rB_sb = ipoolB.tile([128, schedB.nbatch * 8], I16)
                nc.sync.dma_start(gB_sb[:], gB_i[:])
                nc.sync.dma_start(scB_sb[:], scB_i[:])
                nc.sync.dma_start(rB_sb[:], rB_i[:])
                explg = ipoolB.tile([128, schedB.nbatch, 128], BF16)
                for (b0s, bb) in _subcols(schedB.nbatch):
                    nc.gpsimd.dma_gather(
                        explg[:, b0s:b0s + bb, :], expl_table[:],
                        rB_sb[:, b0s * 8:(b0s + bb) * 8],
                        bb * 128, bb * 128, 128,
                        single_packet=False, queue_num=_q())

            # ---------- EA' = (U/D + ef)/D, per half ----------
            if _stop >= 4:
                with tc.tile_pool(name="ea", bufs=2) as eap:
                  for h in range(2):
                   for a0 in range(0, n_a_h[h], 16):
                      aa = min(16, n_a_h[h] - a0)
                      ga = h * n_a_h[0] + a0
                      uc = eap.tile([128, 16, ROW], BF16, tag="uc")
                      efc = eap.tile([128, 16, 128], BF16, tag="efc")
                      nc.sync.dma_start(uc[:, :aa, :], Ured_v[h][:, a0:a0 + aa, :])
                      nc.sync.dma_start(efc[:, :aa, :], ef_v[:, ga:ga + aa, :])
                      dv = eap.tile([128, 16, H], F32, tag="dv")
                      nc.vector.tensor_copy(dv[:, :aa, :], uc[:, :aa, 128:UCOL])
                      nc.vector.tensor_scalar_add(dv[:, :aa, :], dv[:, :aa, :], 1e-30)
                      inv = eap.tile([128, 16, H], F32, tag="inv")
                      nc.vector.reciprocal(inv[:, :aa, :], dv[:, :aa, :])
                      inv_b = inv[:, :aa, :].unsqueeze(3).broadcast_to([128, aa, H, C])
                      t1 = eap.tile([128, 16, 128], F32, tag="t1")
                      nc.vector.tensor_tensor(
                          out=t1[:, :aa, :].rearrange("p a (h c) -> p a h c", h=H),
                          in0=uc[:, :aa, 0:128].rearrange("p a (h c) -> p a h c", h=H),
                          in1=inv_b, op=ALU.mult)
                      nc.vector.tensor_tensor(out=t1[:, :aa, :], in0=t1[:, :aa, :],
                                              in1=efc[:, :aa, :], op=ALU.add)
                      eab = eap.tile([128, 16, 128], BF16, tag="eab")
                      nc.vector.tensor_tensor(
                          out=eab[:, :aa, :].rearrange("p a (h c) -> p a h c", h=H),
                          in0=t1[:, :aa, :].rearrange("p a (h c) -> p a h c", h=H),
                          in1=inv_b, op=ALU.mult)
                      nc.sync.dma_start(EAp_v[:, ga:ga + aa, :], eab[:, :aa, :])
                  nj = ET - E
                  nc.sync.dma_start(EAp[E:ET, :], zbf[0:nj, 0:128])

            if _dbg:
                if _stop >= 1:
                    nc.sync.dma_start(nfw_dbg[:], nfw_table[:])
                if _stop >= 2:
                    for h in range(2):
                        nc.sync.dma_start(U_dbg[h * split:h * split + HBs[h], :], U_half[h][:])
                if _stop >= 3:
                    for h in range(2):
                        nc.sync.dma_start(Ured_dbg[h * split:h * split + HBs[h], :], Ured_half[h][:])
                if _stop >= 4:
                    nc.sync.dma_start(EAp_dbg[:], EAp[:])

            # ---------- pass B ----------
            if _stop >= 5:
                groups_by_tileB = {}
                for g in schedB.groups:
                    groups_by_tileB.setdefault(g[0], []).append(g)
                cur_chunkB = [0]
                stagB = {}
                sc_offB = [0]

                def open_chunkB():
                    stagB["Y"] = spoolB.tile([128, NS, 128], F32, tag="ystag", name="ystag")

                def flush_chunkB():
                    ch = cur_chunkB[0]
                    nb = schedB.chunk_sizes[ch]
                    yst = stagB["Y"]
                    nc.vector.tensor_tensor(
                        out=_ap(yst[:], 0, [[128, nb], [32, H], [1, C]]),
                        in0=_ap(yst[:], 0, [[128, nb], [32, H], [1, C]]),
                        in1=_ap(explg[:], ch * NS * 128, [[128, nb], [1, H], [0, C]]),
                        op=ALU.mult)
                    nc.gpsimd.dma_scatter_add(
                        y[:], yst[:, :nb, :],
                        scB_sb[:, sc_offB[0]:sc_offB[0] + nb * 8],
                        nb * 128, nb * 128, 128,
                        single_packet=False, queue_num=_q())
                    sc_offB[0] += nb * 8
                    cur_chunkB[0] += 1

                open_chunkB()
                for t in range(schedB.ntiles):
                    G = gpoolB.tile([128, TC, 128], BF16, tag="G")
                    for (c0s, cc) in _subcols(TC):
                        nc.gpsimd.dma_gather(
                            G[:, c0s:c0s + cc, :], EAp[:],
                            gB_sb[:, t * TC * 8 + c0s * 8:
                                  t * TC * 8 + (c0s + cc) * 8],
                            cc * 128, cc * 128, 128,
                            single_packet=False, queue_num=_q())
                    for (_, c0, k, B, b0, ch) in groups_by_tileB.get(t, []):
                        if ch != cur_chunkB[0]:
                            flush_chunkB()
                            open_chunkB()
                        bpos = b0 - ch * NS
                        nc.vector.tensor_reduce(
                            out=stagB["Y"][:, bpos:bpos + B, :],
                            in_=_ap(G[:], c0 * 128, [[k * 128, B], [1, 128], [128, k]]),
                            axis=AX.X, op=ALU.add)
                flush_chunkB()
                for p in reversed(pb_stack):
                    p.__exit__(None, None, None)
    nc.compile()
    return nc


def host_inputs(cfg: Cfg, x, ha, W_node, W_edge, attn_l, bias, streams):
    x = np.asarray(x, np.float32)
    ha = np.asarray(ha, np.float32)
    W_node = np.asarray(W_node, np.float32)
    W_edge = np.asarray(W_edge, np.float32)
    attn_flat = np.asarray(attn_l, np.float32).reshape(-1)
    bias = np.asarray(bias, np.float32).reshape(-1)
    attn_rep = np.tile(attn_flat[None, :], (128, 1))
    bias_t = np.tile(bias[None, :], (128, 16))
    ha_pad = np.zeros((cfg.ET_ROWS, cfg.D), np.float32)
    ha_pad[:cfg.E] = ha
    haT = np.ascontiguousarray(ha_pad.T)
    in_maps = []
    for m in range(cfg.NC):
        xs = np.zeros((cfg.NT_ROWS, cfg.D), np.float32)
        xs[:cfg.NSH] = x[m * cfg.NSH:(m + 1) * cfg.NSH]
        st = streams[m]
        in_maps.append({
            "xT": np.ascontiguousarray(xs.T),
            "haT": haT,
            "Wn": W_node, "We": W_edge,
            "attn": attn_rep, "bias_t": bias_t,
            "gA0_i": st["gA0"], "scA0_i": st["scA0"],
            "gA1_i": st["gA1"], "scA1_i": st["scA1"],
            "gB_i": st["gB"], "scB_i": st["scB"], "rB_i": st["rB"],
        })
    return in_maps


# ======================== public entry point ========================
_CFG = Cfg()
LAST_RESULTS = None


def _install_axon_ntff_shim():
    import sys, types, ctypes, contextlib
    import concourse.bass_utils as bu
    bu.upload_artifacts = lambda d: str(d)
    try:
        from antenv.axon_hooks import get_axon_ntff_profile_hook  # noqa
        return
    except ImportError:
        pass
    so_path = "/opt/axon/libaxon_pjrt.so"
    try:
        lib = ctypes.CDLL(so_path)
    except OSError:
        return
    if not hasattr(lib, "axon_start_nrt_profile"):
        return
    lib.axon_start_nrt_profile.argtypes = [ctypes.POINTER(ctypes.c_int64),
                                           ctypes.c_size_t]
    lib.axon_start_nrt_profile.restype = ctypes.c_int64
    lib.axon_stop_nrt_profile.argtypes = [ctypes.c_char_p]
    lib.axon_stop_nrt_profile.restype = ctypes.c_int64

    @contextlib.contextmanager
    def _hook(output_dir, device_ids):
        import jax
        jax.devices()
        if device_ids:
            ids = (ctypes.c_int64 * len(device_ids))(*device_ids)
            rc = lib.axon_start_nrt_profile(ids, len(device_ids))
        else:
            rc = lib.axon_start_nrt_profile(None, 0)
        if rc != 0:
            raise RuntimeError(f"axon_start_nrt_profile rc={rc}")
        try:
            yield
        finally:
            n = lib.axon_stop_nrt_profile(str(output_dir).encode())
            print(f"ntff profile: {n} file(s) -> {output_dir}")

    mod = types.ModuleType("antenv.axon_hooks")
    mod.get_axon_ntff_profile_hook = lambda: _hook
    mod.set_axon_ntff_profile_hook = lambda h: None
    sys.modules["antenv.axon_hooks"] = mod


def kernel(**inputs) -> np.ndarray:
    import os
    from concourse.bass_utils import run_bass_kernel_spmd
    cfg = _CFG
    x = np.asarray(inputs["x"], np.float32)
    ha = np.asarray(inputs["hyperedge_attr"], np.float32)
    node_idx = np.asarray(inputs["node_idx"]).astype(np.int64)
    edge_idx = np.asarray(inputs["edge_idx"]).astype(np.int64)
    schedA, schedB, streams = build_plan(node_idx, edge_idx, cfg)
    nc = build_bass(cfg, schedA, schedB, [list(range(cfg.NC))])
    in_maps = host_inputs(cfg, x, ha, inputs["W_node"], inputs["W_edge"],
                          inputs["attn_l"], inputs["bias"], streams)
    trace = os.environ.get("GNN_TRACE", "0") == "1"
    if trace:
        _install_axon_ntff_shim()
    res = run_bass_kernel_spmd(nc, in_maps, list(range(cfg.NC)), trace=trace)
    global LAST_RESULTS
    LAST_RESULTS = res
    out = np.concatenate(
        [np.asarray(res.results[m]["y"])[:cfg.NSH] for m in range(cfg.NC)], axis=0)
    return np.ascontiguousarray(out, dtype=np.float32)


# revision 47
# speedup vs baseline: 1.0887x; 1.0887x over previous
"""Hypergraph conv kernel, v2.

Pipeline (node-sharded, 8 cores):
  phase1: nfw_table[n] = bf16([exp(a_n)*nf_n (128) | exp(a_n) (4) | pad]),
          expl_table[n] = f32 exp(a_n) (for pass B).
  passA:  per edge-run batch: gather nfw rows, ONE reduce -> [U|D] partial,
          cast bf16, scatter-add into U_table[ET,256] (cols 0:132).
  AR:     one bf16 AllReduce of U_table.
  EA:     EAp[e] = bf16((U/D + ef)/D)   (ef projected during passA window)
  passB:  gather EAp rows per incidence, ONE reduce per run group,
          multiply by expl per chunk, scatter-add into y.

SWDGE gathers/scatters run gen_mode=0: descriptor emission on the Pool
Q7 (~3.3ns/row) is the serial spine; transfers overlap it (engine
releases at doorbell; Tile syncs consumers on the DMA sem).
"""
import numpy as np
from dataclasses import dataclass

import concourse.bass as bass
import concourse.mybir as mybir
import concourse.bacc as bacc
import concourse.tile as tile
import bass_rust
from concourse.library_config import mlp as mlp_library
from concourse._compat import get_trn_type, cdiv

F32 = mybir.dt.float32
BF16 = mybir.dt.bfloat16
I16 = mybir.dt.int16
AX = mybir.AxisListType
ALU = mybir.AluOpType
ACTF = mybir.ActivationFunctionType

ROW = 256          # nfw/U table row width (bf16 elems); cols 0:128 nfw, 128:132 expl
UCOL = 132         # useful cols in nfw/U rows


@dataclass
class Cfg:
    N: int = 100000
    E: int = 25000
    D: int = 128
    H: int = 4
    C: int = 32
    NC: int = 8
    TILE_COLS: int = 30
    NSTAGE: int = 14

    @property
    def NSH(self):
        return self.N // self.NC

    @property
    def NT_ROWS(self):
        return cdiv(self.NSH + 1, 128) * 128

    @property
    def ET_ROWS(self):
        return cdiv(self.E + 1, 128) * 128

    @property
    def DUMMY_NODE(self):
        return self.NSH

    @property
    def JUNK_EDGE(self):
        return self.E


def _runs(keys):
    if len(keys) == 0:
        return (np.zeros(0, np.int64),) * 3
    change = np.flatnonzero(np.diff(keys)) + 1
    starts = np.concatenate([[0], change]).astype(np.int64)
    ends = np.concatenate([change, [len(keys)]]).astype(np.int64)
    return starts, ends - starts, keys[starts].astype(np.int64)


@dataclass
class Sched:
    batches: list          # [(k, tile, c0)]
    groups: list           # [(tile, c0, k, B, b0, chunk)]
    ntiles: int
    nchunks: int
    nbatch: int
    chunk_sizes: list


def _mk_schedule(lens_list, cfg: Cfg) -> Sched:
    sorted_lens = [np.sort(np.asarray(l))[::-1] for l in lens_list]
    nbatch_total = max(cdiv(len(l), 128) for l in sorted_lens)
    batches = []
    for b in range(nbatch_total):
        w = 1
        for ls in sorted_lens:
            if b * 128 < len(ls):
                w = max(w, int(ls[b * 128]))
        batches.append(w)
    assert max(batches) <= cfg.TILE_COLS, \
        f"run length {max(batches)} > TILE_COLS"
    placed = []
    t, c = 0, 0
    for k in batches:
        if c + k > cfg.TILE_COLS:
            t += 1
            c = 0
        placed.append((k, t, c))
        c += k
    ntiles = t + 1 if placed else 1
    nbatch = len(placed)
    nchunks = cdiv(nbatch, cfg.NSTAGE)
    chunk_sizes = [min(cfg.NSTAGE, nbatch - i * cfg.NSTAGE) for i in range(nchunks)]
    groups = []
    for bi, (k, t, c0) in enumerate(placed):
        ch = bi // cfg.NSTAGE
        if groups and groups[-1][0] == t and groups[-1][2] == k \
                and groups[-1][5] == ch \
                and groups[-1][1] + groups[-1][2] * groups[-1][3] == c0 \
                and groups[-1][4] + groups[-1][3] == bi:
            t0, c0g, kg, B, b0, chg = groups[-1]
            groups[-1] = (t0, c0g, kg, B + 1, b0, chg)
        else:
            groups.append((t, c0, k, 1, bi, ch))
    return Sched(placed, groups, ntiles, nchunks, nbatch, chunk_sizes)


def _wrap16(flat):
    assert len(flat) % 16 == 0
    b = flat.reshape(-1, 16).T.astype(np.int16)
    return np.tile(b, (8, 1))


SUBMAX = 30


def _subcols(n):
    return [(i, min(SUBMAX, n - i)) for i in range(0, n, SUBMAX)]


def _mk_streams(sched: Sched, starts, lens, gvals, svals, runvals,
                dummy_g, junk_s, dummy_run, cfg: Cfg):
    TC = cfg.TILE_COLS
    g_arr = np.full((sched.ntiles, TC, 128), dummy_g, np.int64)
    s_arr = np.full((sched.nbatch, 128), junk_s, np.int64)
    r_arr = np.full((sched.nbatch, 128), dummy_run, np.int64)
    order = np.argsort(-lens, kind="stable") if len(lens) else np.zeros(0, np.int64)
    for bi, (k, t, c0) in enumerate(sched.batches):
        idxs = order[bi * 128:(bi + 1) * 128]
        nr = len(idxs)
        if nr:
            st = starts[idxs]
            kr = lens[idxs]
            assert kr[0] <= k
            for kk in np.unique(kr):
                sel = np.flatnonzero(kr == kk)
                gm = gvals[st[sel][None, :] + np.arange(kk)[:, None]]
                g_arr[t, c0:c0 + kk, sel] = gm.T
            s_arr[bi, :nr] = svals[idxs]
            r_arr[bi, :nr] = runvals[idxs]
    g_idx = np.concatenate(
        [_wrap16(g_arr[t, c0:c0 + cc].reshape(-1))
         for t in range(sched.ntiles) for (c0, cc) in _subcols(TC)], axis=1)
    sc_blocks = []
    off = 0
    for nb in sched.chunk_sizes:
        sc_blocks.append(_wrap16(s_arr[off:off + nb].reshape(-1)))
        off += nb
    sc_idx = np.concatenate(sc_blocks, axis=1)
    r_idx = np.concatenate(
        [_wrap16(r_arr[b0:b0 + bb].reshape(-1))
         for (b0, bb) in _subcols(sched.nbatch)], axis=1)
    return g_idx, sc_idx, r_idx


def build_plan(node_idx, edge_idx, cfg: Cfg):
    """Pass A is split into two edge-halves (split at ET_ROWS//2, a slot
    boundary) so each half's partial-U AllReduce can overlap the other
    half's compute. Empty-batch scatter slots target row 0 of the half
    table: they add exact zeros (dummy gathers hit the all-zero nfw row),
    so no junk row is needed."""
    node_idx = np.asarray(node_idx).astype(np.int64)
    edge_idx = np.asarray(edge_idx).astype(np.int64)
    bounds = [(0, cfg.ET_ROWS)]
    percore = []
    for m in range(cfg.NC):
        sel = np.flatnonzero(node_idx // cfg.NSH == m)
        nl = node_idx[sel] - m * cfg.NSH
        eg = edge_idx[sel]
        halves = []
        for (lo, hi) in bounds:
            hs = np.flatnonzero((eg >= lo) & (eg < hi))
            sA, lA, vA = _runs(eg[hs])
            halves.append(dict(nl=nl[hs], sA=sA, lA=lA, vA=vA - lo))
        oB = np.argsort(nl, kind="stable")
        nB = nl[oB]
        eB = eg[oB]
        sB, lB, vB = _runs(nB)
        percore.append(dict(halves=halves, eB=eB, sB=sB, lB=lB, vB=vB))
    schedA = [_mk_schedule([c["halves"][h]["lA"] for c in percore], cfg)
              for h in range(len(bounds))]
    schedB = _mk_schedule([c["lB"] for c in percore], cfg)
    streams = []
    for c in percore:
        st = {}
        for h in range(len(c["halves"])):
            ch = c["halves"][h]
            gA, scA, _ = _mk_streams(
                schedA[h], ch["sA"], ch["lA"],
                gvals=ch["nl"], svals=ch["vA"], runvals=ch["vA"],
                dummy_g=cfg.DUMMY_NODE, junk_s=0,
                dummy_run=0, cfg=cfg)
            st[f"gA{h}"] = gA
            st[f"scA{h}"] = scA
        gB, scB, rB = _mk_streams(
            schedB, c["sB"], c["lB"],
            gvals=c["eB"], svals=c["vB"], runvals=c["vB"],
            dummy_g=cfg.JUNK_EDGE, junk_s=cfg.DUMMY_NODE,
            dummy_run=cfg.DUMMY_NODE, cfg=cfg)
        st.update(gB=gB, scB=scB, rB=rB)
        streams.append(st)
    return schedA, schedB, streams


def _ap(t_ap, off, dims):
    base = t_ap
    part = base.ap[0]
    return bass_rust.AP(base.tensor, base.offset + off, [part] + dims)


def build_bass(cfg: Cfg, schedA: Sched, schedB: Sched, replica_groups):
    import os
    _stops = ["init", "phase1", "passA", "coll", "ea", "full"]
    _stop = _stops.index(os.environ.get("GNN_STOP", "full"))
    TC, NS = cfg.TILE_COLS, cfg.NSTAGE
    H, C = cfg.H, cfg.C
    D = cfg.D
    NT, ET = cfg.NT_ROWS, cfg.ET_ROWS
    NSH, E = cfg.NSH, cfg.E
    n_a_node = NT // 128
    n_a_edge = ET // 128

    nc = bacc.Bacc(get_trn_type() or "TRN2", target_bir_lowering=False, debug=False,
                   num_swdge_queues=4)
    _qrr = [0]

    def _q():
        q = _qrr[0] % 4
        _qrr[0] += 1
        return q

    # ---- I/O ----
    xT = nc.dram_tensor("xT", [D, NT], F32, kind="ExternalInput")
    haT = nc.dram_tensor("haT", [D, ET], F32, kind="ExternalInput")
    Wn = nc.dram_tensor("Wn", [D, H * C], F32, kind="ExternalInput")
    We = nc.dram_tensor("We", [D, H * C], F32, kind="ExternalInput")
    attn = nc.dram_tensor("attn", [128, H * C], F32, kind="ExternalInput")
    bias_t = nc.dram_tensor("bias_t", [128, 16 * H * C], F32, kind="ExternalInput")
    NH = len(schedA)
    split = ET if NH == 1 else (ET // 128 // 2) * 128
    HBs = [split, ET - split][:NH]
    n_a_h = [b // 128 for b in HBs]
    gA_i = [nc.dram_tensor(f"gA{h}_i", [128, schedA[h].ntiles * TC * 8], I16,
                           kind="ExternalInput") for h in range(NH)]
    scA_i = [nc.dram_tensor(f"scA{h}_i", [128, schedA[h].nbatch * 8], I16,
                            kind="ExternalInput") for h in range(NH)]
    gB_i = nc.dram_tensor("gB_i", [128, schedB.ntiles * TC * 8], I16, kind="ExternalInput")
    scB_i = nc.dram_tensor("scB_i", [128, schedB.nbatch * 8], I16, kind="ExternalInput")
    rB_i = nc.dram_tensor("rB_i", [128, schedB.nbatch * 8], I16, kind="ExternalInput")
    y = nc.dram_tensor("y", [NT, H * C], F32, kind="ExternalOutput")
    _dbg = os.environ.get("GNN_DEBUG_OUTS", "0") == "1"
    if _dbg:
        nfw_dbg = nc.dram_tensor("nfw_dbg", [NT, ROW], BF16, kind="ExternalOutput")
        U_dbg = nc.dram_tensor("U_dbg", [ET, ROW], BF16, kind="ExternalOutput")
        Ured_dbg = nc.dram_tensor("Ured_dbg", [ET, ROW], BF16, kind="ExternalOutput")
        EAp_dbg = nc.dram_tensor("EAp_dbg", [ET, 128], BF16, kind="ExternalOutput")

    # ---- internal DRAM ----
    nfw_table = nc.dram_tensor("nfw_table", [NT, ROW], BF16)
    expl_table = nc.dram_tensor("expl_table", [NT, 128], BF16)
    ef_table = nc.dram_tensor("ef_table", [ET, 128], BF16)
    U_half = [nc.dram_tensor(f"U_h{h}", [HBs[h], ROW], BF16) for h in range(NH)]
    Ured_half = [nc.dram_tensor(f"Ured_h{h}", [HBs[h], ROW], BF16, addr_space="Shared")
                 for h in range(NH)]
    EAp = nc.dram_tensor("EAp", [ET, 128], BF16)

    nfw_v = nfw_table[:].rearrange("(a p) c -> p a c", p=128)
    expl_v = expl_table[:].rearrange("(a p) c -> p a c", p=128)
    ef_v = ef_table[:].rearrange("(a p) c -> p a c", p=128)
    U_v = [U_half[h][:].rearrange("(a p) c -> p a c", p=128) for h in range(NH)]
    Ured_v = [Ured_half[h][:].rearrange("(a p) c -> p a c", p=128) for h in range(NH)]
    EAp_v = EAp[:].rearrange("(a p) c -> p a c", p=128)
    y_v = y[:].rearrange("(a p) c -> p a c", p=128)

    # SWDGE gathers/scatters run gen_mode=0: the Pool engine emits
    # descriptors (~3.3ns/row) and releases at doorbell; the DMA-completion
    # sem is attached by Tile, so consumer/WAR sync is fully tracked.
    with tile.TileContext(nc) as tc:
        with tc.tile_pool(name="const", bufs=1) as cpool:
            nc.gpsimd.load_library(mlp_library)
            Wn_sb = cpool.tile([D, H * C], F32)
            We_sb = cpool.tile([D, H * C], F32)
            attn_sb = cpool.tile([128, H * C], F32)
            bias_sb = cpool.tile([128, 16 * H * C], F32)
            zbf = cpool.tile([128, 16 * ROW], BF16)
            nc.sync.dma_start(Wn_sb[:], Wn[:])
            nc.sync.dma_start(We_sb[:], We[:])
            nc.sync.dma_start(attn_sb[:], attn[:])
            nc.sync.dma_start(bias_sb[:], bias_t[:])
            nc.gpsimd.memset(zbf[:], 0.0)

            # ---------- init: U halves = 0 (scalar HWDGE queue) ----------
            zv = zbf[:].rearrange("p (a c) -> p a c", c=ROW)
            for h in range(NH):
                for a0 in range(0, n_a_h[h], 16):
                    aa = min(16, n_a_h[h] - a0)
                    nc.scalar.dma_start(U_v[h][:, a0:a0 + aa, :], zv[:, :aa, :])

            # ---------- phase 1: node projection -> nfw + expl ----------
            if _stop >= 1:
                with (tc.tile_pool(name="p1", bufs=2) as p1,
                    tc.tile_pool(name="p1s", bufs=2) as p1s,
                    tc.tile_pool(name="ps", bufs=4, space="PSUM") as psp):
                  CHK = 16
                  for a0 in range(0, n_a_node, CHK):
                      aa = min(CHK, n_a_node - a0)
                      xc = p1.tile([D, CHK * 128], F32, tag="xc")
                      nc.sync.dma_start(xc[:, :aa * 128], xT[:, a0 * 128:(a0 + aa) * 128])
                      nfst = p1s.tile([128, CHK, 128], F32, tag="nfst")
                      for i in range(aa):
                          mm = psp.tile([128, 128], F32, tag="mm")
                          nc.tensor.matmul(mm[:], xc[:, i * 128:(i + 1) * 128], Wn_sb[:],
                                           start=True, stop=True)
                          nc.vector.tensor_copy(nfst[:, i, :], mm[:])
                      tmp1 = p1s.tile([128, CHK, 128], F32, tag="tmp1")
                      nc.vector.tensor_tensor(
                          out=tmp1[:, :aa, :].rearrange("p a (h c) -> p a h c", h=H),
                          in0=nfst[:, :aa, :].rearrange("p a (h c) -> p a h c", h=H),
                          in1=attn_sb[:].rearrange("p (h c) -> p h c", h=H)
                              .unsqueeze(1).broadcast_to([128, aa, H, C]),
                          op=ALU.mult)
                      praw = p1s.tile([128, CHK, H], F32, tag="praw")
                      nc.vector.tensor_reduce(
                          out=praw[:, :aa, :],
                          in_=tmp1[:, :aa, :].rearrange("p a (h c) -> p a h c", h=H),
                          axis=AX.X, op=ALU.add)
                      estf = p1s.tile([128, CHK, H], F32, tag="estf")
                      nc.scalar.activation(estf[:, :aa, :], praw[:, :aa, :], ACTF.Exp)
                      est = p1s.tile([128, CHK, 128], BF16, tag="est")
                      nc.gpsimd.memset(est[:], 0.0)
                      nc.vector.tensor_copy(est[:, :aa, 0:H], estf[:, :aa, :])
                      nrow = p1s.tile([128, CHK, ROW], BF16, tag="nrow")
                      nc.gpsimd.memset(nrow[:], 0.0)
                      nc.vector.tensor_tensor(
                          out=nrow[:, :aa, 0:128].rearrange("p a (h c) -> p a h c", h=H),
                          in0=nfst[:, :aa, :].rearrange("p a (h c) -> p a h c", h=H),
                          in1=estf[:, :aa, :].unsqueeze(3).broadcast_to([128, aa, H, C]),
                          op=ALU.mult)
                      nc.vector.tensor_copy(nrow[:, :aa, 128:UCOL], estf[:, :aa, :])
                      nc.sync.dma_start(nfw_v[:, a0:a0 + aa, :], nrow[:, :aa, :])
                      nc.sync.dma_start(expl_v[:, a0:a0 + aa, :], est[:, :aa, :])
                  # zero dummy/pad rows (incl. expl cols -> no D contribution)
                  nc.sync.dma_start(nfw_table[NSH:NT, :], zbf[0:NT - NSH, 0:ROW])

            # ---------- edge projection (PE/DVE-light; before passA so its
            # DVE copies don't queue behind passA reduces; ef ready for EA) --
            if _stop >= 1:
                with (tc.tile_pool(name="pe", bufs=2) as pe,
                    tc.tile_pool(name="pes", bufs=2) as pes,
                    tc.tile_pool(name="ps2", bufs=4, space="PSUM") as psp2):
                  CHK = 16
                  for a0 in range(0, n_a_edge, CHK):
                      aa = min(CHK, n_a_edge - a0)
                      hc = pe.tile([D, CHK * 128], F32, tag="hc")
                      nc.sync.dma_start(hc[:, :aa * 128], haT[:, a0 * 128:(a0 + aa) * 128])
                      efst = pes.tile([128, CHK, 128], BF16, tag="efst")
                      for i in range(aa):
                          mm = psp2.tile([128, 128], F32, tag="mm")
                          nc.tensor.matmul(mm[:], hc[:, i * 128:(i + 1) * 128], We_sb[:],
                                           start=True, stop=True)
                          nc.vector.tensor_copy(efst[:, i, :], mm[:])
                      nc.sync.dma_start(ef_v[:, a0:a0 + aa, :], efst[:, :aa, :])

            # ---------- pass A: two edge-halves, AR per half.
            # Each half gets its own pool scope so the collective is
            # emitted OUTSIDE any pool (inside-pool collectives deadlock
            # Tile's barrier machinery), yet between the halves on the
            # Pool queue so AR(h0) flies during half 1's compute. ----------
            if _stop >= 2:
                  for h in range(NH):
                    with (tc.tile_pool(name=f"gA{h}", bufs=4) as gpool,
                        tc.tile_pool(name=f"stA{h}", bufs=4) as spool,
                        tc.tile_pool(name=f"idxA{h}", bufs=1) as ipool):
                      gA_sb = [None, None]
                      scA_sb = [None, None]
                      gA_sb[h] = ipool.tile([128, schedA[h].ntiles * TC * 8], I16,
                                            name=f"gAsb{h}")
                      scA_sb[h] = ipool.tile([128, schedA[h].nbatch * 8], I16,
                                             name=f"scAsb{h}")
                      nc.sync.dma_start(gA_sb[h][:], gA_i[h][:])
                      nc.sync.dma_start(scA_sb[h][:], scA_i[h][:])
                      sched = schedA[h]
                      groups_by_tile = {}
                      for g in sched.groups:
                          groups_by_tile.setdefault(g[0], []).append(g)

                      cur_chunk = [0]
                      stag = {}
                      sc_off = [0]

                      def open_chunk():
                          stag["U"] = spool.tile([128, NS, UCOL], F32,
                                                 tag="ustag", name="ustag")

                      def flush_chunk(h=h, sched=sched):
                          ch = cur_chunk[0]
                          nb = sched.chunk_sizes[ch]
                          # full 512B rows (proven 256B-multiple scatter
                          # geometry); pads add zero, zeroed once per
                          # buffer instance and never dirtied after.
                          ubf = spool.tile([128, NS, ROW], BF16,
                                           tag="ubf", name="ubf")
                          if ch < 4:
                              nc.gpsimd.memset(ubf[:, :, UCOL:], 0.0)
                          nc.vector.tensor_copy(ubf[:, :nb, 0:UCOL],
                                                stag["U"][:, :nb, :])
                          nc.gpsimd.dma_scatter_add(
                              U_half[h][:], ubf[:, :nb, :],
                              scA_sb[h][:, sc_off[0]:sc_off[0] + nb * 8],
                              nb * 128, nb * 128, ROW,
                              single_packet=False, queue_num=_q())
                          sc_off[0] += nb * 8
                          cur_chunk[0] += 1

                      open_chunk()
                      for t in range(sched.ntiles):
                          G = gpool.tile([128, TC, ROW], BF16, tag="G")
                          for (c0s, cc) in _subcols(TC):
                              nc.gpsimd.dma_gather(
                                  G[:, c0s:c0s + cc, :], nfw_table[:],
                                  gA_sb[h][:, t * TC * 8 + c0s * 8:
                                        t * TC * 8 + (c0s + cc) * 8],
                                  cc * 128, cc * 128, ROW,
                                  single_packet=False, queue_num=_q())
                          for (_, c0, k, B, b0, ch) in groups_by_tile.get(t, []):
                              if ch != cur_chunk[0]:
                                  flush_chunk()
                                  open_chunk()
                              bpos = b0 - ch * NS
                              nc.vector.tensor_reduce(
                                  out=stag["U"][:, bpos:bpos + B, :],
                                  in_=_ap(G[:], c0 * ROW,
                                          [[k * ROW, B], [1, UCOL], [ROW, k]]),
                                  axis=AX.X, op=ALU.add)
                      flush_chunk()
                    if _stop >= 3:
                        # emitted after this half's pool scope closes, before
                        # the next half's — AR(h) overlaps half h+1 compute
                        nc.gpsimd.collective_compute(
                            "AllReduce", ALU.add,
                            replica_groups=replica_groups,
                            ins=[U_half[h][:]], outs=[Ured_half[h][:]])

            # ---------- y = bias init (needed only by pass B) ----------
            bv = bias_sb[:].rearrange("p (a c) -> p a c", c=H * C)
            for a0 in range(0, n_a_node, 16):
                aa = min(16, n_a_node - a0)
                nc.scalar.dma_start(y_v[:, a0:a0 + aa, :], bv[:, :aa, :])

            # ---------- pass B prologue: index/expl loads (overlap AR) ----------
            if _stop >= 5:
                pb_stack = [tc.tile_pool(name="gB", bufs=6),
                            tc.tile_pool(name="stB", bufs=4),
                            tc.tile_pool(name="idxB", bufs=1)]
                gpoolB, spoolB, ipoolB = [p.__enter__() for p in pb_stack]
                gB_sb = ipoolB.tile([128, schedB.ntiles * TC * 8], I16)
                scB_sb = ipoolB.tile([128, schedB.nbatch * 8], I16)
                rB_sb = ipoolB.tile([128, schedB.nbatch * 8], I16)
                nc.sync.dma_start(gB_sb[:], gB_i[:])
                nc.sync.dma_start(scB_sb[:], scB_i[:])
                nc.sync.dma_start(rB_sb[:], rB_i[:])
                explg = ipoolB.tile([128, schedB.nbatch, 128], BF16)
                for (b0s, bb) in _subcols(schedB.nbatch):
                    nc.gpsimd.dma_gather(
                        explg[:, b0s:b0s + bb, :], expl_table[:],
                        rB_sb[:, b0s * 8:(b0s + bb) * 8],
                        bb * 128, bb * 128, 128,
                        single_packet=False, queue_num=_q())

            # ---------- EA' = (U/D + ef)/D, per half ----------
            if _stop >= 4:
                with tc.tile_pool(name="ea", bufs=2) as eap:
                  for h in range(NH):
                   for a0 in range(0, n_a_h[h], 16):
                      aa = min(16, n_a_h[h] - a0)
                      ga = h * n_a_h[0] + a0
                      uc = eap.tile([128, 16, ROW], BF16, tag="uc")
                      efc = eap.tile([128, 16, 128], BF16, tag="efc")
                      nc.sync.dma_start(uc[:, :aa, :], Ured_v[h][:, a0:a0 + aa, :])
                      nc.sync.dma_start(efc[:, :aa, :], ef_v[:, ga:ga + aa, :])
                      dv = eap.tile([128, 16, H], F32, tag="dv")
                      nc.vector.tensor_copy(dv[:, :aa, :], uc[:, :aa, 128:UCOL])
                      nc.vector.tensor_scalar_add(dv[:, :aa, :], dv[:, :aa, :], 1e-30)
                      inv = eap.tile([128, 16, H], F32, tag="inv")
                      nc.vector.reciprocal(inv[:, :aa, :], dv[:, :aa, :])
                      inv_b = inv[:, :aa, :].unsqueeze(3).broadcast_to([128, aa, H, C])
                      t1 = eap.tile([128, 16, 128], F32, tag="t1")
                      nc.vector.tensor_tensor(
                          out=t1[:, :aa, :].rearrange("p a (h c) -> p a h c", h=H),
                          in0=uc[:, :aa, 0:128].rearrange("p a (h c) -> p a h c", h=H),
                          in1=inv_b, op=ALU.mult)
                      nc.vector.tensor_tensor(out=t1[:, :aa, :], in0=t1[:, :aa, :],
                                              in1=efc[:, :aa, :], op=ALU.add)
                      eab = eap.tile([128, 16, 128], BF16, tag="eab")
                      nc.vector.tensor_tensor(
                          out=eab[:, :aa, :].rearrange("p a (h c) -> p a h c", h=H),
                          in0=t1[:, :aa, :].rearrange("p a (h c) -> p a h c", h=H),
                          in1=inv_b, op=ALU.mult)
                      nc.sync.dma_start(EAp_v[:, ga:ga + aa, :], eab[:, :aa, :])
                  nj = ET - E
                  nc.sync.dma_start(EAp[E:ET, :], zbf[0:nj, 0:128])

            if _dbg:
                if _stop >= 1:
                    nc.sync.dma_start(nfw_dbg[:], nfw_table[:])
                if _stop >= 2:
                    for h in range(NH):
                        nc.sync.dma_start(U_dbg[h * split:h * split + HBs[h], :], U_half[h][:])
                if _stop >= 3:
                    for h in range(NH):
                        nc.sync.dma_start(Ured_dbg[h * split:h * split + HBs[h], :], Ured_half[h][:])
                if _stop >= 4:
                    nc.sync.dma_start(EAp_dbg[:], EAp[:])

            # ---------- pass B ----------
            if _stop >= 5:
                groups_by_tileB = {}
                for g in schedB.groups:
                    groups_by_tileB.setdefault(g[0], []).append(g)
                cur_chunkB = [0]
                stagB = {}
                sc_offB = [0]

                def open_chunkB():
                    stagB["Y"] = spoolB.tile([128, NS, 128], F32, tag="ystag", name="ystag")

                def flush_chunkB():
                    ch = cur_chunkB[0]
                    nb = schedB.chunk_sizes[ch]
                    yst = stagB["Y"]
                    nc.vector.tensor_tensor(
                        out=_ap(yst[:], 0, [[128, nb], [32, H], [1, C]]),
                        in0=_ap(yst[:], 0, [[128, nb], [32, H], [1, C]]),
                        in1=_ap(explg[:], ch * NS * 128, [[128, nb], [1, H], [0, C]]),
                        op=ALU.mult)
                    nc.gpsimd.dma_scatter_add(
                        y[:], yst[:, :nb, :],
                        scB_sb[:, sc_offB[0]:sc_offB[0] + nb * 8],
                        nb * 128, nb * 128, 128,
                        single_packet=False, queue_num=_q())
                    sc_offB[0] += nb * 8
                    cur_chunkB[0] += 1

                open_chunkB()
                for t in range(schedB.ntiles):
                    G = gpoolB.tile([128, TC, 128], BF16, tag="G")
                    for (c0s, cc) in _subcols(TC):
                        nc.gpsimd.dma_gather(
                            G[:, c0s:c0s + cc, :], EAp[:],
                            gB_sb[:, t * TC * 8 + c0s * 8:
                                  t * TC * 8 + (c0s + cc) * 8],
                            cc * 128, cc * 128, 128,
                            single_packet=False, queue_num=_q())
                    for (_, c0, k, B, b0, ch) in groups_by_tileB.get(t, []):
                        if ch != cur_chunkB[0]:
                            flush_chunkB()
                            open_chunkB()
                        bpos = b0 - ch * NS
                        nc.vector.tensor_reduce(
                            out=stagB["Y"][:, bpos:bpos + B, :],
                            in_=_ap(G[:], c0 * 128, [[k * 128, B], [1, 128], [128, k]]),
                            axis=AX.X, op=ALU.add)
                flush_chunkB()
                for p in reversed(pb_stack):
                    p.__exit__(None, None, None)
    nc.compile()
    return nc


def host_inputs(cfg: Cfg, x, ha, W_node, W_edge, attn_l, bias, streams):
    x = np.asarray(x, np.float32)
    ha = np.asarray(ha, np.float32)
    W_node = np.asarray(W_node, np.float32)
    W_edge = np.asarray(W_edge, np.float32)
    attn_flat = np.asarray(attn_l, np.float32).reshape(-1)
    bias = np.asarray(bias, np.float32).reshape(-1)
    attn_rep = np.tile(attn_flat[None, :], (128, 1))
    bias_t = np.tile(bias[None, :], (128, 16))
    ha_pad = np.zeros((cfg.ET_ROWS, cfg.D), np.float32)
    ha_pad[:cfg.E] = ha
    haT = np.ascontiguousarray(ha_pad.T)
    in_maps = []
    for m in range(cfg.NC):
        xs = np.zeros((cfg.NT_ROWS, cfg.D), np.float32)
        xs[:cfg.NSH] = x[m * cfg.NSH:(m + 1) * cfg.NSH]
        st = streams[m]
        in_maps.append({
            "xT": np.ascontiguousarray(xs.T),
            "haT": haT,
            "Wn": W_node, "We": W_edge,
            "attn": attn_rep, "bias_t": bias_t,
            "gB_i": st["gB"], "scB_i": st["scB"], "rB_i": st["rB"],
            **{f"{k}_i": v for k, v in st.items()
               if k.startswith(("gA", "scA"))},
        })
    return in_maps


# ======================== public entry point ========================
_CFG = Cfg()
LAST_RESULTS = None


def _install_axon_ntff_shim():
    import sys, types, ctypes, contextlib
    import concourse.bass_utils as bu
    bu.upload_artifacts = lambda d: str(d)
    try:
        from antenv.axon_hooks import get_axon_ntff_profile_hook  # noqa
        return
    except ImportError:
        pass
    so_path = "/opt/axon/libaxon_pjrt.so"
    try:
        lib = ctypes.CDLL(so_path)
    except OSError:
        return
    if not hasattr(lib, "axon_start_nrt_profile"):
        return
    lib.axon_start_nrt_profile.argtypes = [ctypes.POINTER(ctypes.c_int64),
                                           ctypes.c_size_t]
    lib.axon_start_nrt_profile.restype = ctypes.c_int64
    lib.axon_stop_nrt_profile.argtypes = [ctypes.c_char_p]
    lib.axon_stop_nrt_profile.restype = ctypes.c_int64

    @contextlib.contextmanager
    def _hook(output_dir, device_ids):
        import jax
        jax.devices()
        if device_ids:
            ids = (ctypes.c_int64 * len(device_ids))(*device_ids)
            rc = lib.axon_start_nrt_profile(ids, len(device_ids))
        else:
            rc = lib.axon_start_nrt_profile(None, 0)
        if rc != 0:
            raise RuntimeError(f"axon_start_nrt_profile rc={rc}")
        try:
            yield
        finally:
            n = lib.axon_stop_nrt_profile(str(output_dir).encode())
            print(f"ntff profile: {n} file(s) -> {output_dir}")

    mod = types.ModuleType("antenv.axon_hooks")
    mod.get_axon_ntff_profile_hook = lambda: _hook
    mod.set_axon_ntff_profile_hook = lambda h: None
    sys.modules["antenv.axon_hooks"] = mod


def kernel(**inputs) -> np.ndarray:
    import os
    from concourse.bass_utils import run_bass_kernel_spmd
    cfg = _CFG
    x = np.asarray(inputs["x"], np.float32)
    ha = np.asarray(inputs["hyperedge_attr"], np.float32)
    node_idx = np.asarray(inputs["node_idx"]).astype(np.int64)
    edge_idx = np.asarray(inputs["edge_idx"]).astype(np.int64)
    schedA, schedB, streams = build_plan(node_idx, edge_idx, cfg)
    nc = build_bass(cfg, schedA, schedB, [list(range(cfg.NC))])
    in_maps = host_inputs(cfg, x, ha, inputs["W_node"], inputs["W_edge"],
                          inputs["attn_l"], inputs["bias"], streams)
    trace = os.environ.get("GNN_TRACE", "0") == "1"
    if trace:
        _install_axon_ntff_shim()
    res = run_bass_kernel_spmd(nc, in_maps, list(range(cfg.NC)), trace=trace)
    global LAST_RESULTS
    LAST_RESULTS = res
    out = np.concatenate(
        [np.asarray(res.results[m]["y"])[:cfg.NSH] for m in range(cfg.NC)], axis=0)
    return np.ascontiguousarray(out, dtype=np.float32)


# revision 48
# speedup vs baseline: 1.2048x; 1.1066x over previous
"""Hypergraph conv kernel, v2.

Pipeline (node-sharded, 8 cores):
  phase1: nfw_table[n] = bf16([exp(a_n)*nf_n (128) | exp(a_n) (4) | pad]),
          expl_table[n] = f32 exp(a_n) (for pass B).
  passA:  per edge-run batch: gather nfw rows, ONE reduce -> [U|D] partial,
          cast bf16, scatter-add into U_table[ET,256] (cols 0:132).
  AR:     one bf16 AllReduce of U_table.
  EA:     EAp[e] = bf16((U/D + ef)/D)   (ef projected during passA window)
  passB:  gather EAp rows per incidence, ONE reduce per run group,
          multiply by expl per chunk, scatter-add into y.

SWDGE gathers/scatters run gen_mode=0: descriptor emission on the Pool
Q7 (~3.3ns/row) is the serial spine; transfers overlap it (engine
releases at doorbell; Tile syncs consumers on the DMA sem).
"""
import numpy as np
from dataclasses import dataclass

import concourse.bass as bass
import concourse.mybir as mybir
import concourse.bacc as bacc
import concourse.tile as tile
import bass_rust
from concourse.library_config import mlp as mlp_library
from concourse._compat import get_trn_type, cdiv

F32 = mybir.dt.float32
BF16 = mybir.dt.bfloat16
I16 = mybir.dt.int16
AX = mybir.AxisListType
ALU = mybir.AluOpType
ACTF = mybir.ActivationFunctionType

ROW = 256          # nfw/U table row width (bf16 elems); cols 0:128 nfw, 128:132 expl
UCOL = 132         # useful cols in nfw/U rows


@dataclass
class Cfg:
    N: int = 100000
    E: int = 25000
    D: int = 128
    H: int = 4
    C: int = 32
    NC: int = 8
    TILE_COLS: int = 30
    NSTAGE: int = 21

    @property
    def NSH(self):
        return self.N // self.NC

    @property
    def NT_ROWS(self):
        return cdiv(self.NSH + 1, 128) * 128

    @property
    def ET_ROWS(self):
        return cdiv(self.E + 1, 128) * 128

    @property
    def DUMMY_NODE(self):
        return self.NSH

    @property
    def JUNK_EDGE(self):
        return self.E


def _runs(keys):
    if len(keys) == 0:
        return (np.zeros(0, np.int64),) * 3
    change = np.flatnonzero(np.diff(keys)) + 1
    starts = np.concatenate([[0], change]).astype(np.int64)
    ends = np.concatenate([change, [len(keys)]]).astype(np.int64)
    return starts, ends - starts, keys[starts].astype(np.int64)


@dataclass
class Sched:
    batches: list          # [(k, tile, c0)]
    groups: list           # [(tile, c0, k, B, b0, chunk)]
    ntiles: int
    nchunks: int
    nbatch: int
    chunk_sizes: list


def _mk_schedule(lens_list, cfg: Cfg) -> Sched:
    sorted_lens = [np.sort(np.asarray(l))[::-1] for l in lens_list]
    nbatch_total = max(cdiv(len(l), 128) for l in sorted_lens)
    batches = []
    for b in range(nbatch_total):
        w = 1
        for ls in sorted_lens:
            if b * 128 < len(ls):
                w = max(w, int(ls[b * 128]))
        batches.append(w)
    assert max(batches) <= cfg.TILE_COLS, \
        f"run length {max(batches)} > TILE_COLS"
    placed = []
    t, c = 0, 0
    for k in batches:
        if c + k > cfg.TILE_COLS:
            t += 1
            c = 0
        placed.append((k, t, c))
        c += k
    ntiles = t + 1 if placed else 1
    nbatch = len(placed)
    nchunks = cdiv(nbatch, cfg.NSTAGE)
    chunk_sizes = [min(cfg.NSTAGE, nbatch - i * cfg.NSTAGE) for i in range(nchunks)]
    groups = []
    for bi, (k, t, c0) in enumerate(placed):
        ch = bi // cfg.NSTAGE
        if groups and groups[-1][0] == t and groups[-1][2] == k \
                and groups[-1][5] == ch \
                and groups[-1][1] + groups[-1][2] * groups[-1][3] == c0 \
                and groups[-1][4] + groups[-1][3] == bi:
            t0, c0g, kg, B, b0, chg = groups[-1]
            groups[-1] = (t0, c0g, kg, B + 1, b0, chg)
        else:
            groups.append((t, c0, k, 1, bi, ch))
    return Sched(placed, groups, ntiles, nchunks, nbatch, chunk_sizes)


def _wrap16(flat):
    assert len(flat) % 16 == 0
    b = flat.reshape(-1, 16).T.astype(np.int16)
    return np.tile(b, (8, 1))


SUBMAX = 30


def _subcols(n):
    return [(i, min(SUBMAX, n - i)) for i in range(0, n, SUBMAX)]


def _mk_streams(sched: Sched, starts, lens, gvals, svals, runvals,
                dummy_g, junk_s, dummy_run, cfg: Cfg):
    TC = cfg.TILE_COLS
    g_arr = np.full((sched.ntiles, TC, 128), dummy_g, np.int64)
    s_arr = np.full((sched.nbatch, 128), junk_s, np.int64)
    r_arr = np.full((sched.nbatch, 128), dummy_run, np.int64)
    order = np.argsort(-lens, kind="stable") if len(lens) else np.zeros(0, np.int64)
    for bi, (k, t, c0) in enumerate(sched.batches):
        idxs = order[bi * 128:(bi + 1) * 128]
        nr = len(idxs)
        if nr:
            st = starts[idxs]
            kr = lens[idxs]
            assert kr[0] <= k
            for kk in np.unique(kr):
                sel = np.flatnonzero(kr == kk)
                gm = gvals[st[sel][None, :] + np.arange(kk)[:, None]]
                g_arr[t, c0:c0 + kk, sel] = gm.T
            s_arr[bi, :nr] = svals[idxs]
            r_arr[bi, :nr] = runvals[idxs]
    g_idx = np.concatenate(
        [_wrap16(g_arr[t, c0:c0 + cc].reshape(-1))
         for t in range(sched.ntiles) for (c0, cc) in _subcols(TC)], axis=1)
    sc_blocks = []
    off = 0
    for nb in sched.chunk_sizes:
        sc_blocks.append(_wrap16(s_arr[off:off + nb].reshape(-1)))
        off += nb
    sc_idx = np.concatenate(sc_blocks, axis=1)
    r_idx = np.concatenate(
        [_wrap16(r_arr[b0:b0 + bb].reshape(-1))
         for (b0, bb) in _subcols(sched.nbatch)], axis=1)
    return g_idx, sc_idx, r_idx


def build_plan(node_idx, edge_idx, cfg: Cfg):
    """Pass A is split into two edge-halves (split at ET_ROWS//2, a slot
    boundary) so each half's partial-U AllReduce can overlap the other
    half's compute. Empty-batch scatter slots target row 0 of the half
    table: they add exact zeros (dummy gathers hit the all-zero nfw row),
    so no junk row is needed."""
    node_idx = np.asarray(node_idx).astype(np.int64)
    edge_idx = np.asarray(edge_idx).astype(np.int64)
    bounds = [(0, cfg.ET_ROWS)]
    percore = []
    for m in range(cfg.NC):
        sel = np.flatnonzero(node_idx // cfg.NSH == m)
        nl = node_idx[sel] - m * cfg.NSH
        eg = edge_idx[sel]
        halves = []
        for (lo, hi) in bounds:
            hs = np.flatnonzero((eg >= lo) & (eg < hi))
            sA, lA, vA = _runs(eg[hs])
            halves.append(dict(nl=nl[hs], sA=sA, lA=lA, vA=vA - lo))
        oB = np.argsort(nl, kind="stable")
        nB = nl[oB]
        eB = eg[oB]
        sB, lB, vB = _runs(nB)
        percore.append(dict(halves=halves, eB=eB, sB=sB, lB=lB, vB=vB))
    schedA = [_mk_schedule([c["halves"][h]["lA"] for c in percore], cfg)
              for h in range(len(bounds))]
    schedB = _mk_schedule([c["lB"] for c in percore], cfg)
    streams = []
    for c in percore:
        st = {}
        for h in range(len(c["halves"])):
            ch = c["halves"][h]
            gA, scA, _ = _mk_streams(
                schedA[h], ch["sA"], ch["lA"],
                gvals=ch["nl"], svals=ch["vA"], runvals=ch["vA"],
                dummy_g=cfg.DUMMY_NODE, junk_s=0,
                dummy_run=0, cfg=cfg)
            st[f"gA{h}"] = gA
            st[f"scA{h}"] = scA
        gB, scB, rB = _mk_streams(
            schedB, c["sB"], c["lB"],
            gvals=c["eB"], svals=c["vB"], runvals=c["vB"],
            dummy_g=cfg.JUNK_EDGE, junk_s=cfg.DUMMY_NODE,
            dummy_run=cfg.DUMMY_NODE, cfg=cfg)
        st.update(gB=gB, scB=scB, rB=rB)
        streams.append(st)
    return schedA, schedB, streams


def _ap(t_ap, off, dims):
    base = t_ap
    part = base.ap[0]
    return bass_rust.AP(base.tensor, base.offset + off, [part] + dims)


def build_bass(cfg: Cfg, schedA: Sched, schedB: Sched, replica_groups):
    import os
    _stops = ["init", "phase1", "passA", "coll", "ea", "full"]
    _stop = _stops.index(os.environ.get("GNN_STOP", "full"))
    TC, NS = cfg.TILE_COLS, cfg.NSTAGE
    H, C = cfg.H, cfg.C
    D = cfg.D
    NT, ET = cfg.NT_ROWS, cfg.ET_ROWS
    NSH, E = cfg.NSH, cfg.E
    n_a_node = NT // 128
    n_a_edge = ET // 128

    nc = bacc.Bacc(get_trn_type() or "TRN2", target_bir_lowering=False, debug=False,
                   num_swdge_queues=4)
    _qrr = [0]

    def _q():
        q = _qrr[0] % 4
        _qrr[0] += 1
        return q

    # ---- I/O ----
    xT = nc.dram_tensor("xT", [D, NT], F32, kind="ExternalInput")
    haT = nc.dram_tensor("haT", [D, ET], F32, kind="ExternalInput")
    Wn = nc.dram_tensor("Wn", [D, H * C], F32, kind="ExternalInput")
    We = nc.dram_tensor("We", [D, H * C], F32, kind="ExternalInput")
    attn = nc.dram_tensor("attn", [128, H * C], F32, kind="ExternalInput")
    bias_t = nc.dram_tensor("bias_t", [128, 16 * H * C], F32, kind="ExternalInput")
    NH = len(schedA)
    split = ET if NH == 1 else (ET // 128 // 2) * 128
    HBs = [split, ET - split][:NH]
    n_a_h = [b // 128 for b in HBs]
    gA_i = [nc.dram_tensor(f"gA{h}_i", [128, schedA[h].ntiles * TC * 8], I16,
                           kind="ExternalInput") for h in range(NH)]
    scA_i = [nc.dram_tensor(f"scA{h}_i", [128, schedA[h].nbatch * 8], I16,
                            kind="ExternalInput") for h in range(NH)]
    gB_i = nc.dram_tensor("gB_i", [128, schedB.ntiles * TC * 8], I16, kind="ExternalInput")
    scB_i = nc.dram_tensor("scB_i", [128, schedB.nbatch * 8], I16, kind="ExternalInput")
    rB_i = nc.dram_tensor("rB_i", [128, schedB.nbatch * 8], I16, kind="ExternalInput")
    y = nc.dram_tensor("y", [NT, H * C], F32, kind="ExternalOutput")
    _dbg = os.environ.get("GNN_DEBUG_OUTS", "0") == "1"
    if _dbg:
        nfw_dbg = nc.dram_tensor("nfw_dbg", [NT, ROW], BF16, kind="ExternalOutput")
        U_dbg = nc.dram_tensor("U_dbg", [ET, ROW], BF16, kind="ExternalOutput")
        Ured_dbg = nc.dram_tensor("Ured_dbg", [ET, ROW], BF16, kind="ExternalOutput")
        EAp_dbg = nc.dram_tensor("EAp_dbg", [ET, 128], BF16, kind="ExternalOutput")

    # ---- internal DRAM ----
    nfw_table = nc.dram_tensor("nfw_table", [NT, ROW], BF16)
    expl_table = nc.dram_tensor("expl_table", [NT, 128], BF16)
    ef_table = nc.dram_tensor("ef_table", [ET, 128], BF16)
    U_half = [nc.dram_tensor(f"U_h{h}", [HBs[h], ROW], BF16) for h in range(NH)]
    Ured_half = [nc.dram_tensor(f"Ured_h{h}", [HBs[h], ROW], BF16, addr_space="Shared")
                 for h in range(NH)]
    EAp = nc.dram_tensor("EAp", [ET, 128], BF16)

    nfw_v = nfw_table[:].rearrange("(a p) c -> p a c", p=128)
    expl_v = expl_table[:].rearrange("(a p) c -> p a c", p=128)
    ef_v = ef_table[:].rearrange("(a p) c -> p a c", p=128)
    U_v = [U_half[h][:].rearrange("(a p) c -> p a c", p=128) for h in range(NH)]
    Ured_v = [Ured_half[h][:].rearrange("(a p) c -> p a c", p=128) for h in range(NH)]
    EAp_v = EAp[:].rearrange("(a p) c -> p a c", p=128)
    y_v = y[:].rearrange("(a p) c -> p a c", p=128)

    # SWDGE gathers/scatters run gen_mode=0: the Pool engine emits
    # descriptors (~3.3ns/row) and releases at doorbell; the DMA-completion
    # sem is attached by Tile, so consumer/WAR sync is fully tracked.
    with tile.TileContext(nc) as tc:
        with tc.tile_pool(name="const", bufs=1) as cpool:
            nc.gpsimd.load_library(mlp_library)
            Wn_sb = cpool.tile([D, H * C], F32)
            We_sb = cpool.tile([D, H * C], F32)
            attn_sb = cpool.tile([128, H * C], F32)
            bias_sb = cpool.tile([128, 16 * H * C], F32)
            zbf = cpool.tile([128, 16 * ROW], BF16)
            nc.sync.dma_start(Wn_sb[:], Wn[:])
            nc.sync.dma_start(We_sb[:], We[:])
            nc.sync.dma_start(attn_sb[:], attn[:])
            nc.sync.dma_start(bias_sb[:], bias_t[:])
            nc.gpsimd.memset(zbf[:], 0.0)

            # ---------- init: U halves = 0 (scalar HWDGE queue) ----------
            zv = zbf[:].rearrange("p (a c) -> p a c", c=ROW)
            for h in range(NH):
                for a0 in range(0, n_a_h[h], 16):
                    aa = min(16, n_a_h[h] - a0)
                    nc.scalar.dma_start(U_v[h][:, a0:a0 + aa, :], zv[:, :aa, :])

            # ---------- phase 1: node projection -> nfw + expl ----------
            if _stop >= 1:
                with (tc.tile_pool(name="p1", bufs=2) as p1,
                    tc.tile_pool(name="p1s", bufs=2) as p1s,
                    tc.tile_pool(name="ps", bufs=4, space="PSUM") as psp):
                  CHK = 16
                  for a0 in range(0, n_a_node, CHK):
                      aa = min(CHK, n_a_node - a0)
                      xc = p1.tile([D, CHK * 128], F32, tag="xc")
                      nc.sync.dma_start(xc[:, :aa * 128], xT[:, a0 * 128:(a0 + aa) * 128])
                      nfst = p1s.tile([128, CHK, 128], F32, tag="nfst")
                      for i in range(aa):
                          mm = psp.tile([128, 128], F32, tag="mm")
                          nc.tensor.matmul(mm[:], xc[:, i * 128:(i + 1) * 128], Wn_sb[:],
                                           start=True, stop=True)
                          nc.vector.tensor_copy(nfst[:, i, :], mm[:])
                      tmp1 = p1s.tile([128, CHK, 128], F32, tag="tmp1")
                      nc.vector.tensor_tensor(
                          out=tmp1[:, :aa, :].rearrange("p a (h c) -> p a h c", h=H),
                          in0=nfst[:, :aa, :].rearrange("p a (h c) -> p a h c", h=H),
                          in1=attn_sb[:].rearrange("p (h c) -> p h c", h=H)
                              .unsqueeze(1).broadcast_to([128, aa, H, C]),
                          op=ALU.mult)
                      praw = p1s.tile([128, CHK, H], F32, tag="praw")
                      nc.vector.tensor_reduce(
                          out=praw[:, :aa, :],
                          in_=tmp1[:, :aa, :].rearrange("p a (h c) -> p a h c", h=H),
                          axis=AX.X, op=ALU.add)
                      estf = p1s.tile([128, CHK, H], F32, tag="estf")
                      nc.scalar.activation(estf[:, :aa, :], praw[:, :aa, :], ACTF.Exp)
                      est = p1s.tile([128, CHK, 128], BF16, tag="est")
                      nc.gpsimd.memset(est[:], 0.0)
                      nc.vector.tensor_copy(est[:, :aa, 0:H], estf[:, :aa, :])
                      nrow = p1s.tile([128, CHK, ROW], BF16, tag="nrow")
                      nc.gpsimd.memset(nrow[:], 0.0)
                      nc.vector.tensor_tensor(
                          out=nrow[:, :aa, 0:128].rearrange("p a (h c) -> p a h c", h=H),
                          in0=nfst[:, :aa, :].rearrange("p a (h c) -> p a h c", h=H),
                          in1=estf[:, :aa, :].unsqueeze(3).broadcast_to([128, aa, H, C]),
                          op=ALU.mult)
                      nc.vector.tensor_copy(nrow[:, :aa, 128:UCOL], estf[:, :aa, :])
                      nc.sync.dma_start(nfw_v[:, a0:a0 + aa, :], nrow[:, :aa, :])
                      nc.sync.dma_start(expl_v[:, a0:a0 + aa, :], est[:, :aa, :])
                  # zero dummy/pad rows (incl. expl cols -> no D contribution)
                  nc.sync.dma_start(nfw_table[NSH:NT, :], zbf[0:NT - NSH, 0:ROW])

            # ---------- edge projection (PE/DVE-light; before passA so its
            # DVE copies don't queue behind passA reduces; ef ready for EA) --
            if _stop >= 1:
                with (tc.tile_pool(name="pe", bufs=2) as pe,
                    tc.tile_pool(name="pes", bufs=2) as pes,
                    tc.tile_pool(name="ps2", bufs=4, space="PSUM") as psp2):
                  CHK = 16
                  for a0 in range(0, n_a_edge, CHK):
                      aa = min(CHK, n_a_edge - a0)
                      hc = pe.tile([D, CHK * 128], F32, tag="hc")
                      nc.sync.dma_start(hc[:, :aa * 128], haT[:, a0 * 128:(a0 + aa) * 128])
                      efst = pes.tile([128, CHK, 128], BF16, tag="efst")
                      for i in range(aa):
                          mm = psp2.tile([128, 128], F32, tag="mm")
                          nc.tensor.matmul(mm[:], hc[:, i * 128:(i + 1) * 128], We_sb[:],
                                           start=True, stop=True)
                          nc.vector.tensor_copy(efst[:, i, :], mm[:])
                      nc.sync.dma_start(ef_v[:, a0:a0 + aa, :], efst[:, :aa, :])

            # ---------- pass A: two edge-halves, AR per half.
            # Each half gets its own pool scope so the collective is
            # emitted OUTSIDE any pool (inside-pool collectives deadlock
            # Tile's barrier machinery), yet between the halves on the
            # Pool queue so AR(h0) flies during half 1's compute. ----------
            if _stop >= 2:
                  for h in range(NH):
                    with (tc.tile_pool(name=f"gA{h}", bufs=4) as gpool,
                        tc.tile_pool(name=f"stA{h}", bufs=3) as spool,
                        tc.tile_pool(name=f"idxA{h}", bufs=1) as ipool):
                      gA_sb = [None, None]
                      scA_sb = [None, None]
                      gA_sb[h] = ipool.tile([128, schedA[h].ntiles * TC * 8], I16,
                                            name=f"gAsb{h}")
                      scA_sb[h] = ipool.tile([128, schedA[h].nbatch * 8], I16,
                                             name=f"scAsb{h}")
                      nc.sync.dma_start(gA_sb[h][:], gA_i[h][:])
                      nc.sync.dma_start(scA_sb[h][:], scA_i[h][:])
                      sched = schedA[h]
                      groups_by_tile = {}
                      for g in sched.groups:
                          groups_by_tile.setdefault(g[0], []).append(g)

                      cur_chunk = [0]
                      stag = {}
                      sc_off = [0]
                      pend = []

                      def open_chunk():
                          stag["U"] = spool.tile([128, NS, UCOL], F32,
                                                 tag="ustag", name="ustag")

                      def emit_scatter(ubf, nb, off, h=h):
                          nc.gpsimd.dma_scatter_add(
                              U_half[h][:], ubf[:, :nb, :],
                              scA_sb[h][:, off:off + nb * 8],
                              nb * 128, nb * 128, ROW,
                              single_packet=False, queue_num=_q())

                      def flush_chunk(h=h, sched=sched):
                          ch = cur_chunk[0]
                          nb = sched.chunk_sizes[ch]
                          # full 512B rows (proven 256B-multiple scatter
                          # geometry); pads add zero, zeroed once per
                          # buffer instance and never dirtied after.
                          ubf = spool.tile([128, NS, ROW], BF16,
                                           tag="ubf", name="ubf")
                          if ch < 3:
                              nc.gpsimd.memset(ubf[:, :, UCOL:], 0.0)
                          nc.vector.tensor_copy(ubf[:, :nb, 0:UCOL],
                                                stag["U"][:, :nb, :])
                          # defer the scatter one chunk so its dispatch
                          # never stalls the Pool queue on the cast
                          pend.append((ubf, nb, sc_off[0]))
                          if len(pend) > 1:
                              emit_scatter(*pend.pop(0))
                          sc_off[0] += nb * 8
                          cur_chunk[0] += 1

                      open_chunk()
                      for t in range(sched.ntiles):
                          G = gpool.tile([128, TC, ROW], BF16, tag="G")
                          for (c0s, cc) in _subcols(TC):
                              nc.gpsimd.dma_gather(
                                  G[:, c0s:c0s + cc, :], nfw_table[:],
                                  gA_sb[h][:, t * TC * 8 + c0s * 8:
                                        t * TC * 8 + (c0s + cc) * 8],
                                  cc * 128, cc * 128, ROW,
                                  single_packet=False, queue_num=_q())
                          for (_, c0, k, B, b0, ch) in groups_by_tile.get(t, []):
                              if ch != cur_chunk[0]:
                                  flush_chunk()
                                  open_chunk()
                              bpos = b0 - ch * NS
                              nc.vector.tensor_reduce(
                                  out=stag["U"][:, bpos:bpos + B, :],
                                  in_=_ap(G[:], c0 * ROW,
                                          [[k * ROW, B], [1, UCOL], [ROW, k]]),
                                  axis=AX.X, op=ALU.add)
                      flush_chunk()
                      while pend:
                          emit_scatter(*pend.pop(0))
                    if _stop >= 3:
                        # emitted after this half's pool scope closes, before
                        # the next half's — AR(h) overlaps half h+1 compute
                        nc.gpsimd.collective_compute(
                            "AllReduce", ALU.add,
                            replica_groups=replica_groups,
                            ins=[U_half[h][:]], outs=[Ured_half[h][:]])

            # ---------- y = bias init (needed only by pass B) ----------
            bv = bias_sb[:].rearrange("p (a c) -> p a c", c=H * C)
            for a0 in range(0, n_a_node, 16):
                aa = min(16, n_a_node - a0)
                nc.scalar.dma_start(y_v[:, a0:a0 + aa, :], bv[:, :aa, :])

            # ---------- pass B prologue: index/expl loads (overlap AR) ----------
            if _stop >= 5:
                pb_stack = [tc.tile_pool(name="gB", bufs=6),
                            tc.tile_pool(name="stB", bufs=4),
                            tc.tile_pool(name="idxB", bufs=1)]
                gpoolB, spoolB, ipoolB = [p.__enter__() for p in pb_stack]
                gB_sb = ipoolB.tile([128, schedB.ntiles * TC * 8], I16)
                scB_sb = ipoolB.tile([128, schedB.nbatch * 8], I16)
                rB_sb = ipoolB.tile([128, schedB.nbatch * 8], I16)
                nc.sync.dma_start(gB_sb[:], gB_i[:])
                nc.sync.dma_start(scB_sb[:], scB_i[:])
                nc.sync.dma_start(rB_sb[:], rB_i[:])
                explg = ipoolB.tile([128, schedB.nbatch, 128], BF16)
                for (b0s, bb) in _subcols(schedB.nbatch):
                    nc.gpsimd.dma_gather(
                        explg[:, b0s:b0s + bb, :], expl_table[:],
                        rB_sb[:, b0s * 8:(b0s + bb) * 8],
                        bb * 128, bb * 128, 128,
                        single_packet=False, queue_num=_q())

            # ---------- EA' = (U/D + ef)/D, per half ----------
            if _stop >= 4:
                with tc.tile_pool(name="ea", bufs=2) as eap:
                  for h in range(NH):
                   for a0 in range(0, n_a_h[h], 16):
                      aa = min(16, n_a_h[h] - a0)
                      ga = h * n_a_h[0] + a0
                      uc = eap.tile([128, 16, ROW], BF16, tag="uc")
                      efc = eap.tile([128, 16, 128], BF16, tag="efc")
                      nc.sync.dma_start(uc[:, :aa, :], Ured_v[h][:, a0:a0 + aa, :])
                      nc.sync.dma_start(efc[:, :aa, :], ef_v[:, ga:ga + aa, :])
                      dv = eap.tile([128, 16, H], F32, tag="dv")
                      nc.vector.tensor_copy(dv[:, :aa, :], uc[:, :aa, 128:UCOL])
                      nc.vector.tensor_scalar_add(dv[:, :aa, :], dv[:, :aa, :], 1e-30)
                      inv = eap.tile([128, 16, H], F32, tag="inv")
                      nc.vector.reciprocal(inv[:, :aa, :], dv[:, :aa, :])
                      inv_b = inv[:, :aa, :].unsqueeze(3).broadcast_to([128, aa, H, C])
                      t1 = eap.tile([128, 16, 128], F32, tag="t1")
                      nc.vector.tensor_tensor(
                          out=t1[:, :aa, :].rearrange("p a (h c) -> p a h c", h=H),
                          in0=uc[:, :aa, 0:128].rearrange("p a (h c) -> p a h c", h=H),
                          in1=inv_b, op=ALU.mult)
                      nc.vector.tensor_tensor(out=t1[:, :aa, :], in0=t1[:, :aa, :],
                                              in1=efc[:, :aa, :], op=ALU.add)
                      eab = eap.tile([128, 16, 128], BF16, tag="eab")
                      nc.vector.tensor_tensor(
                          out=eab[:, :aa, :].rearrange("p a (h c) -> p a h c", h=H),
                          in0=t1[:, :aa, :].rearrange("p a (h c) -> p a h c", h=H),
                          in1=inv_b, op=ALU.mult)
                      nc.sync.dma_start(EAp_v[:, ga:ga + aa, :], eab[:, :aa, :])
                  nj = ET - E
                  nc.sync.dma_start(EAp[E:ET, :], zbf[0:nj, 0:128])

            if _dbg:
                if _stop >= 1:
                    nc.sync.dma_start(nfw_dbg[:], nfw_table[:])
                if _stop >= 2:
                    for h in range(NH):
                        nc.sync.dma_start(U_dbg[h * split:h * split + HBs[h], :], U_half[h][:])
                if _stop >= 3:
                    for h in range(NH):
                        nc.sync.dma_start(Ured_dbg[h * split:h * split + HBs[h], :], Ured_half[h][:])
                if _stop >= 4:
                    nc.sync.dma_start(EAp_dbg[:], EAp[:])

            # ---------- pass B ----------
            if _stop >= 5:
                groups_by_tileB = {}
                for g in schedB.groups:
                    groups_by_tileB.setdefault(g[0], []).append(g)
                cur_chunkB = [0]
                stagB = {}
                sc_offB = [0]

                def open_chunkB():
                    stagB["Y"] = spoolB.tile([128, NS, 128], F32, tag="ystag", name="ystag")

                pendB = []

                def emit_scatterB(yst, nb, off):
                    nc.gpsimd.dma_scatter_add(
                        y[:], yst[:, :nb, :],
                        scB_sb[:, off:off + nb * 8],
                        nb * 128, nb * 128, 128,
                        single_packet=False, queue_num=_q())

                def flush_chunkB():
                    ch = cur_chunkB[0]
                    nb = schedB.chunk_sizes[ch]
                    yst = stagB["Y"]
                    nc.vector.tensor_tensor(
                        out=_ap(yst[:], 0, [[128, nb], [32, H], [1, C]]),
                        in0=_ap(yst[:], 0, [[128, nb], [32, H], [1, C]]),
                        in1=_ap(explg[:], ch * NS * 128, [[128, nb], [1, H], [0, C]]),
                        op=ALU.mult)
                    pendB.append((yst, nb, sc_offB[0]))
                    if len(pendB) > 1:
                        emit_scatterB(*pendB.pop(0))
                    sc_offB[0] += nb * 8
                    cur_chunkB[0] += 1

                open_chunkB()
                for t in range(schedB.ntiles):
                    G = gpoolB.tile([128, TC, 128], BF16, tag="G")
                    for (c0s, cc) in _subcols(TC):
                        nc.gpsimd.dma_gather(
                            G[:, c0s:c0s + cc, :], EAp[:],
                            gB_sb[:, t * TC * 8 + c0s * 8:
                                  t * TC * 8 + (c0s + cc) * 8],
                            cc * 128, cc * 128, 128,
                            single_packet=False, queue_num=_q())
                    for (_, c0, k, B, b0, ch) in groups_by_tileB.get(t, []):
                        if ch != cur_chunkB[0]:
                            flush_chunkB()
                            open_chunkB()
                        bpos = b0 - ch * NS
                        nc.vector.tensor_reduce(
                            out=stagB["Y"][:, bpos:bpos + B, :],
                            in_=_ap(G[:], c0 * 128, [[k * 128, B], [1, 128], [128, k]]),
                            axis=AX.X, op=ALU.add)
                flush_chunkB()
                while pendB:
                    emit_scatterB(*pendB.pop(0))
                for p in reversed(pb_stack):
                    p.__exit__(None, None, None)
    nc.compile()
    return nc


def host_inputs(cfg: Cfg, x, ha, W_node, W_edge, attn_l, bias, streams):
    x = np.asarray(x, np.float32)
    ha = np.asarray(ha, np.float32)
    W_node = np.asarray(W_node, np.float32)
    W_edge = np.asarray(W_edge, np.float32)
    attn_flat = np.asarray(attn_l, np.float32).reshape(-1)
    bias = np.asarray(bias, np.float32).reshape(-1)
    attn_rep = np.tile(attn_flat[None, :], (128, 1))
    bias_t = np.tile(bias[None, :], (128, 16))
    ha_pad = np.zeros((cfg.ET_ROWS, cfg.D), np.float32)
    ha_pad[:cfg.E] = ha
    haT = np.ascontiguousarray(ha_pad.T)
    in_maps = []
    for m in range(cfg.NC):
        xs = np.zeros((cfg.NT_ROWS, cfg.D), np.float32)
        xs[:cfg.NSH] = x[m * cfg.NSH:(m + 1) * cfg.NSH]
        st = streams[m]
        in_maps.append({
            "xT": np.ascontiguousarray(xs.T),
            "haT": haT,
            "Wn": W_node, "We": W_edge,
            "attn": attn_rep, "bias_t": bias_t,
            "gB_i": st["gB"], "scB_i": st["scB"], "rB_i": st["rB"],
            **{f"{k}_i": v for k, v in st.items()
               if k.startswith(("gA", "scA"))},
        })
    return in_maps


# ======================== public entry point ========================
_CFG = Cfg()
LAST_RESULTS = None


def _install_axon_ntff_shim():
    import sys, types, ctypes, contextlib
    import concourse.bass_utils as bu
    bu.upload_artifacts = lambda d: str(d)
    try:
        from antenv.axon_hooks import get_axon_ntff_profile_hook  # noqa
        return
    except ImportError:
        pass
    so_path = "/opt/axon/libaxon_pjrt.so"
    try:
        lib = ctypes.CDLL(so_path)
    except OSError:
        return
    if not hasattr(lib, "axon_start_nrt_profile"):
        return
    lib.axon_start_nrt_profile.argtypes = [ctypes.POINTER(ctypes.c_int64),
                                           ctypes.c_size_t]
    lib.axon_start_nrt_profile.restype = ctypes.c_int64
    lib.axon_stop_nrt_profile.argtypes = [ctypes.c_char_p]
    lib.axon_stop_nrt_profile.restype = ctypes.c_int64

    @contextlib.contextmanager
    def _hook(output_dir, device_ids):
        import jax
        jax.devices()
        if device_ids:
            ids = (ctypes.c_int64 * len(device_ids))(*device_ids)
            rc = lib.axon_start_nrt_profile(ids, len(device_ids))
        else:
            rc = lib.axon_start_nrt_profile(None, 0)
        if rc != 0:
            raise RuntimeError(f"axon_start_nrt_profile rc={rc}")
        try:
            yield
        finally:
            n = lib.axon_stop_nrt_profile(str(output_dir).encode())
            print(f"ntff profile: {n} file(s) -> {output_dir}")

    mod = types.ModuleType("antenv.axon_hooks")
    mod.get_axon_ntff_profile_hook = lambda: _hook
    mod.set_axon_ntff_profile_hook = lambda h: None
    sys.modules["antenv.axon_hooks"] = mod


def kernel(**inputs) -> np.ndarray:
    import os
    from concourse.bass_utils import run_bass_kernel_spmd
    cfg = _CFG
    x = np.asarray(inputs["x"], np.float32)
    ha = np.asarray(inputs["hyperedge_attr"], np.float32)
    node_idx = np.asarray(inputs["node_idx"]).astype(np.int64)
    edge_idx = np.asarray(inputs["edge_idx"]).astype(np.int64)
    schedA, schedB, streams = build_plan(node_idx, edge_idx, cfg)
    nc = build_bass(cfg, schedA, schedB, [list(range(cfg.NC))])
    in_maps = host_inputs(cfg, x, ha, inputs["W_node"], inputs["W_edge"],
                          inputs["attn_l"], inputs["bias"], streams)
    trace = os.environ.get("GNN_TRACE", "0") == "1"
    if trace:
        _install_axon_ntff_shim()
    res = run_bass_kernel_spmd(nc, in_maps, list(range(cfg.NC)), trace=trace)
    global LAST_RESULTS
    LAST_RESULTS = res
    out = np.concatenate(
        [np.asarray(res.results[m]["y"])[:cfg.NSH] for m in range(cfg.NC)], axis=0)
    return np.ascontiguousarray(out, dtype=np.float32)


# revision 49
# speedup vs baseline: 1.2431x; 1.0318x over previous
"""Hypergraph conv kernel, v2.

Pipeline (node-sharded, 8 cores):
  phase1: nfw_table[n] = bf16([exp(a_n)*nf_n (128) | exp(a_n) (4) | pad]),
          expl_table[n] = f32 exp(a_n) (for pass B).
  passA:  per edge-run batch: gather nfw rows, ONE reduce -> [U|D] partial,
          cast bf16, scatter-add into U_table[ET,256] (cols 0:132).
  AR:     one bf16 AllReduce of U_table.
  EA:     EAp[e] = bf16((U/D + ef)/D)   (ef projected during passA window)
  passB:  gather EAp rows per incidence, ONE reduce per run group,
          multiply by expl per chunk, scatter-add into y.

SWDGE gathers/scatters run gen_mode=0: descriptor emission on the Pool
Q7 (~3.3ns/row) is the serial spine; transfers overlap it (engine
releases at doorbell; Tile syncs consumers on the DMA sem).
"""
import numpy as np
from dataclasses import dataclass

import concourse.bass as bass
import concourse.mybir as mybir
import concourse.bacc as bacc
import concourse.tile as tile
import bass_rust
from concourse.library_config import mlp as mlp_library
from concourse._compat import get_trn_type, cdiv

F32 = mybir.dt.float32
BF16 = mybir.dt.bfloat16
I16 = mybir.dt.int16
AX = mybir.AxisListType
ALU = mybir.AluOpType
ACTF = mybir.ActivationFunctionType

ROW = 256          # nfw/U table row width (bf16 elems); cols 0:128 nfw, 128:132 expl
UCOL = 132         # useful cols in nfw/U rows


@dataclass
class Cfg:
    N: int = 100000
    E: int = 25000
    D: int = 128
    H: int = 4
    C: int = 32
    NC: int = 8
    TILE_COLS: int = 30
    NSTAGE: int = 21

    @property
    def NSH(self):
        return self.N // self.NC

    @property
    def NT_ROWS(self):
        return cdiv(self.NSH + 1, 128) * 128

    @property
    def ET_ROWS(self):
        return cdiv(self.E + 1, 128) * 128

    @property
    def DUMMY_NODE(self):
        return self.NSH

    @property
    def JUNK_EDGE(self):
        return self.E


def _runs(keys):
    if len(keys) == 0:
        return (np.zeros(0, np.int64),) * 3
    change = np.flatnonzero(np.diff(keys)) + 1
    starts = np.concatenate([[0], change]).astype(np.int64)
    ends = np.concatenate([change, [len(keys)]]).astype(np.int64)
    return starts, ends - starts, keys[starts].astype(np.int64)


@dataclass
class Sched:
    batches: list          # [(k, tile, c0)]
    groups: list           # [(tile, c0, k, B, b0, chunk)]
    ntiles: int
    nchunks: int
    nbatch: int
    chunk_sizes: list


def _mk_schedule(lens_list, cfg: Cfg) -> Sched:
    sorted_lens = [np.sort(np.asarray(l))[::-1] for l in lens_list]
    nbatch_total = max(cdiv(len(l), 128) for l in sorted_lens)
    batches = []
    for b in range(nbatch_total):
        w = 1
        for ls in sorted_lens:
            if b * 128 < len(ls):
                w = max(w, int(ls[b * 128]))
        batches.append(w)
    assert max(batches) <= cfg.TILE_COLS, \
        f"run length {max(batches)} > TILE_COLS"
    placed = []
    t, c = 0, 0
    for k in batches:
        if c + k > cfg.TILE_COLS:
            t += 1
            c = 0
        placed.append((k, t, c))
        c += k
    ntiles = t + 1 if placed else 1
    nbatch = len(placed)
    nchunks = cdiv(nbatch, cfg.NSTAGE)
    chunk_sizes = [min(cfg.NSTAGE, nbatch - i * cfg.NSTAGE) for i in range(nchunks)]
    groups = []
    for bi, (k, t, c0) in enumerate(placed):
        ch = bi // cfg.NSTAGE
        if groups and groups[-1][0] == t and groups[-1][2] == k \
                and groups[-1][5] == ch \
                and groups[-1][1] + groups[-1][2] * groups[-1][3] == c0 \
                and groups[-1][4] + groups[-1][3] == bi:
            t0, c0g, kg, B, b0, chg = groups[-1]
            groups[-1] = (t0, c0g, kg, B + 1, b0, chg)
        else:
            groups.append((t, c0, k, 1, bi, ch))
    return Sched(placed, groups, ntiles, nchunks, nbatch, chunk_sizes)


def _wrap16(flat):
    assert len(flat) % 16 == 0
    b = flat.reshape(-1, 16).T.astype(np.int16)
    return np.tile(b, (8, 1))


SUBMAX = 30


def _subcols(n):
    return [(i, min(SUBMAX, n - i)) for i in range(0, n, SUBMAX)]


def _mk_streams(sched: Sched, starts, lens, gvals, svals, runvals,
                dummy_g, junk_s, dummy_run, cfg: Cfg):
    TC = cfg.TILE_COLS
    g_arr = np.full((sched.ntiles, TC, 128), dummy_g, np.int64)
    s_arr = np.full((sched.nbatch, 128), junk_s, np.int64)
    r_arr = np.full((sched.nbatch, 128), dummy_run, np.int64)
    order = np.argsort(-lens, kind="stable") if len(lens) else np.zeros(0, np.int64)
    for bi, (k, t, c0) in enumerate(sched.batches):
        idxs = order[bi * 128:(bi + 1) * 128]
        nr = len(idxs)
        if nr:
            st = starts[idxs]
            kr = lens[idxs]
            assert kr[0] <= k
            for kk in np.unique(kr):
                sel = np.flatnonzero(kr == kk)
                gm = gvals[st[sel][None, :] + np.arange(kk)[:, None]]
                g_arr[t, c0:c0 + kk, sel] = gm.T
            s_arr[bi, :nr] = svals[idxs]
            r_arr[bi, :nr] = runvals[idxs]
    g_idx = np.concatenate(
        [_wrap16(g_arr[t, c0:c0 + cc].reshape(-1))
         for t in range(sched.ntiles) for (c0, cc) in _subcols(TC)], axis=1)
    sc_blocks = []
    off = 0
    for nb in sched.chunk_sizes:
        sc_blocks.append(_wrap16(s_arr[off:off + nb].reshape(-1)))
        off += nb
    sc_idx = np.concatenate(sc_blocks, axis=1)
    r_idx = np.concatenate(
        [_wrap16(r_arr[b0:b0 + bb].reshape(-1))
         for (b0, bb) in _subcols(sched.nbatch)], axis=1)
    return g_idx, sc_idx, r_idx


def build_plan(node_idx, edge_idx, cfg: Cfg):
    """Pass A is split into two edge-halves (split at ET_ROWS//2, a slot
    boundary) so each half's partial-U AllReduce can overlap the other
    half's compute. Empty-batch scatter slots target row 0 of the half
    table: they add exact zeros (dummy gathers hit the all-zero nfw row),
    so no junk row is needed."""
    node_idx = np.asarray(node_idx).astype(np.int64)
    edge_idx = np.asarray(edge_idx).astype(np.int64)
    bounds = [(0, cfg.ET_ROWS)]
    percore = []
    for m in range(cfg.NC):
        sel = np.flatnonzero(node_idx // cfg.NSH == m)
        nl = node_idx[sel] - m * cfg.NSH
        eg = edge_idx[sel]
        halves = []
        for (lo, hi) in bounds:
            hs = np.flatnonzero((eg >= lo) & (eg < hi))
            sA, lA, vA = _runs(eg[hs])
            halves.append(dict(nl=nl[hs], sA=sA, lA=lA, vA=vA - lo))
        oB = np.argsort(nl, kind="stable")
        nB = nl[oB]
        eB = eg[oB]
        sB, lB, vB = _runs(nB)
        percore.append(dict(halves=halves, eB=eB, sB=sB, lB=lB, vB=vB))
    schedA = [_mk_schedule([c["halves"][h]["lA"] for c in percore], cfg)
              for h in range(len(bounds))]
    schedB = _mk_schedule([c["lB"] for c in percore], cfg)
    streams = []
    for c in percore:
        st = {}
        for h in range(len(c["halves"])):
            ch = c["halves"][h]
            gA, scA, _ = _mk_streams(
                schedA[h], ch["sA"], ch["lA"],
                gvals=ch["nl"], svals=ch["vA"], runvals=ch["vA"],
                dummy_g=cfg.DUMMY_NODE, junk_s=0,
                dummy_run=0, cfg=cfg)
            st[f"gA{h}"] = gA
            st[f"scA{h}"] = scA
        gB, scB, rB = _mk_streams(
            schedB, c["sB"], c["lB"],
            gvals=c["eB"], svals=c["vB"], runvals=c["vB"],
            dummy_g=cfg.JUNK_EDGE, junk_s=cfg.DUMMY_NODE,
            dummy_run=cfg.DUMMY_NODE, cfg=cfg)
        st.update(gB=gB, scB=scB, rB=rB)
        streams.append(st)
    return schedA, schedB, streams


def _ap(t_ap, off, dims):
    base = t_ap
    part = base.ap[0]
    return bass_rust.AP(base.tensor, base.offset + off, [part] + dims)


def build_bass(cfg: Cfg, schedA: Sched, schedB: Sched, replica_groups):
    import os
    _stops = ["init", "phase1", "passA", "coll", "ea", "full"]
    _stop = _stops.index(os.environ.get("GNN_STOP", "full"))
    TC, NS = cfg.TILE_COLS, cfg.NSTAGE
    H, C = cfg.H, cfg.C
    D = cfg.D
    NT, ET = cfg.NT_ROWS, cfg.ET_ROWS
    NSH, E = cfg.NSH, cfg.E
    n_a_node = NT // 128
    n_a_edge = ET // 128

    nc = bacc.Bacc(get_trn_type() or "TRN2", target_bir_lowering=False, debug=False,
                   num_swdge_queues=4)
    _qrr = [0]

    def _q():
        q = _qrr[0] % 4
        _qrr[0] += 1
        return q

    # ---- I/O ----
    xT = nc.dram_tensor("xT", [D, NT], F32, kind="ExternalInput")
    haT = nc.dram_tensor("haT", [D, ET], F32, kind="ExternalInput")
    Wn = nc.dram_tensor("Wn", [D, H * C], F32, kind="ExternalInput")
    We = nc.dram_tensor("We", [D, H * C], F32, kind="ExternalInput")
    attn = nc.dram_tensor("attn", [128, H * C], F32, kind="ExternalInput")
    bias_t = nc.dram_tensor("bias_t", [128, 16 * H * C], F32, kind="ExternalInput")
    NH = len(schedA)
    split = ET if NH == 1 else (ET // 128 // 2) * 128
    HBs = [split, ET - split][:NH]
    n_a_h = [b // 128 for b in HBs]
    gA_i = [nc.dram_tensor(f"gA{h}_i", [128, schedA[h].ntiles * TC * 8], I16,
                           kind="ExternalInput") for h in range(NH)]
    scA_i = [nc.dram_tensor(f"scA{h}_i", [128, schedA[h].nbatch * 8], I16,
                            kind="ExternalInput") for h in range(NH)]
    gB_i = nc.dram_tensor("gB_i", [128, schedB.ntiles * TC * 8], I16, kind="ExternalInput")
    scB_i = nc.dram_tensor("scB_i", [128, schedB.nbatch * 8], I16, kind="ExternalInput")
    rB_i = nc.dram_tensor("rB_i", [128, schedB.nbatch * 8], I16, kind="ExternalInput")
    y = nc.dram_tensor("y", [NT, H * C], F32, kind="ExternalOutput")
    _dbg = os.environ.get("GNN_DEBUG_OUTS", "0") == "1"
    if _dbg:
        nfw_dbg = nc.dram_tensor("nfw_dbg", [NT, ROW], BF16, kind="ExternalOutput")
        U_dbg = nc.dram_tensor("U_dbg", [ET, ROW], BF16, kind="ExternalOutput")
        Ured_dbg = nc.dram_tensor("Ured_dbg", [ET, ROW], BF16, kind="ExternalOutput")
        EAp_dbg = nc.dram_tensor("EAp_dbg", [ET, 128], BF16, kind="ExternalOutput")

    # ---- internal DRAM ----
    nfw_table = nc.dram_tensor("nfw_table", [NT, ROW], BF16)
    expl_table = nc.dram_tensor("expl_table", [NT, 128], BF16)
    ef_table = nc.dram_tensor("ef_table", [ET, 128], BF16)
    U_half = [nc.dram_tensor(f"U_h{h}", [HBs[h], ROW], BF16) for h in range(NH)]
    Ured_half = [nc.dram_tensor(f"Ured_h{h}", [HBs[h], ROW], BF16, addr_space="Shared")
                 for h in range(NH)]
    EAp = nc.dram_tensor("EAp", [ET, 128], BF16)

    nfw_v = nfw_table[:].rearrange("(a p) c -> p a c", p=128)
    expl_v = expl_table[:].rearrange("(a p) c -> p a c", p=128)
    ef_v = ef_table[:].rearrange("(a p) c -> p a c", p=128)
    U_v = [U_half[h][:].rearrange("(a p) c -> p a c", p=128) for h in range(NH)]
    Ured_v = [Ured_half[h][:].rearrange("(a p) c -> p a c", p=128) for h in range(NH)]
    EAp_v = EAp[:].rearrange("(a p) c -> p a c", p=128)
    y_v = y[:].rearrange("(a p) c -> p a c", p=128)

    # SWDGE gathers/scatters run gen_mode=0: the Pool engine emits
    # descriptors (~3.3ns/row) and releases at doorbell; the DMA-completion
    # sem is attached by Tile, so consumer/WAR sync is fully tracked.
    with tile.TileContext(nc) as tc:
        with tc.tile_pool(name="const", bufs=1) as cpool:
            nc.gpsimd.load_library(mlp_library)
            Wn_sb = cpool.tile([D, H * C], F32)
            We_sb = cpool.tile([D, H * C], F32)
            attn_sb = cpool.tile([128, H * C], F32)
            bias_sb = cpool.tile([128, 16 * H * C], F32)
            zbf = cpool.tile([128, 16 * ROW], BF16)
            nc.sync.dma_start(Wn_sb[:], Wn[:])
            nc.sync.dma_start(We_sb[:], We[:])
            nc.sync.dma_start(attn_sb[:], attn[:])
            nc.sync.dma_start(bias_sb[:], bias_t[:])
            nc.gpsimd.memset(zbf[:], 0.0)

            # ---------- init: U halves = 0 (scalar HWDGE queue) ----------
            zv = zbf[:].rearrange("p (a c) -> p a c", c=ROW)
            for h in range(NH):
                for a0 in range(0, n_a_h[h], 16):
                    aa = min(16, n_a_h[h] - a0)
                    nc.scalar.dma_start(U_v[h][:, a0:a0 + aa, :], zv[:, :aa, :])

            # ---------- phase 1: node projection -> nfw + expl ----------
            if _stop >= 1:
                with (tc.tile_pool(name="p1", bufs=2) as p1,
                    tc.tile_pool(name="p1s", bufs=2) as p1s,
                    tc.tile_pool(name="ps", bufs=4, space="PSUM") as psp):
                  CHK = 16
                  for a0 in range(0, n_a_node, CHK):
                      aa = min(CHK, n_a_node - a0)
                      xc = p1.tile([D, CHK * 128], F32, tag="xc")
                      nc.sync.dma_start(xc[:, :aa * 128], xT[:, a0 * 128:(a0 + aa) * 128])
                      nfst = p1s.tile([128, CHK, 128], F32, tag="nfst")
                      for i in range(aa):
                          mm = psp.tile([128, 128], F32, tag="mm")
                          nc.tensor.matmul(mm[:], xc[:, i * 128:(i + 1) * 128], Wn_sb[:],
                                           start=True, stop=True)
                          nc.vector.tensor_copy(nfst[:, i, :], mm[:])
                      tmp1 = p1s.tile([128, CHK, 128], F32, tag="tmp1")
                      nc.vector.tensor_tensor(
                          out=tmp1[:, :aa, :].rearrange("p a (h c) -> p a h c", h=H),
                          in0=nfst[:, :aa, :].rearrange("p a (h c) -> p a h c", h=H),
                          in1=attn_sb[:].rearrange("p (h c) -> p h c", h=H)
                              .unsqueeze(1).broadcast_to([128, aa, H, C]),
                          op=ALU.mult)
                      praw = p1s.tile([128, CHK, H], F32, tag="praw")
                      nc.vector.tensor_reduce(
                          out=praw[:, :aa, :],
                          in_=tmp1[:, :aa, :].rearrange("p a (h c) -> p a h c", h=H),
                          axis=AX.X, op=ALU.add)
                      estf = p1s.tile([128, CHK, H], F32, tag="estf")
                      nc.scalar.activation(estf[:, :aa, :], praw[:, :aa, :], ACTF.Exp)
                      est = p1s.tile([128, CHK, 128], BF16, tag="est")
                      nc.gpsimd.memset(est[:], 0.0)
                      nc.vector.tensor_copy(est[:, :aa, 0:H], estf[:, :aa, :])
                      nrow = p1s.tile([128, CHK, ROW], BF16, tag="nrow")
                      nc.gpsimd.memset(nrow[:], 0.0)
                      nc.vector.tensor_tensor(
                          out=nrow[:, :aa, 0:128].rearrange("p a (h c) -> p a h c", h=H),
                          in0=nfst[:, :aa, :].rearrange("p a (h c) -> p a h c", h=H),
                          in1=estf[:, :aa, :].unsqueeze(3).broadcast_to([128, aa, H, C]),
                          op=ALU.mult)
                      nc.vector.tensor_copy(nrow[:, :aa, 128:UCOL], estf[:, :aa, :])
                      nc.sync.dma_start(nfw_v[:, a0:a0 + aa, :], nrow[:, :aa, :])
                      nc.sync.dma_start(expl_v[:, a0:a0 + aa, :], est[:, :aa, :])
                  # zero dummy/pad rows (incl. expl cols -> no D contribution)
                  nc.sync.dma_start(nfw_table[NSH:NT, :], zbf[0:NT - NSH, 0:ROW])

            # ---------- pass A: two edge-halves, AR per half.
            # Each half gets its own pool scope so the collective is
            # emitted OUTSIDE any pool (inside-pool collectives deadlock
            # Tile's barrier machinery), yet between the halves on the
            # Pool queue so AR(h0) flies during half 1's compute. ----------
            if _stop >= 2:
                  for h in range(NH):
                    with (tc.tile_pool(name=f"gA{h}", bufs=4) as gpool,
                        tc.tile_pool(name=f"stA{h}", bufs=3) as spool,
                        tc.tile_pool(name=f"idxA{h}", bufs=1) as ipool):
                      gA_sb = [None, None]
                      scA_sb = [None, None]
                      gA_sb[h] = ipool.tile([128, schedA[h].ntiles * TC * 8], I16,
                                            name=f"gAsb{h}")
                      scA_sb[h] = ipool.tile([128, schedA[h].nbatch * 8], I16,
                                             name=f"scAsb{h}")
                      nc.sync.dma_start(gA_sb[h][:], gA_i[h][:])
                      nc.sync.dma_start(scA_sb[h][:], scA_i[h][:])
                      sched = schedA[h]
                      groups_by_tile = {}
                      for g in sched.groups:
                          groups_by_tile.setdefault(g[0], []).append(g)

                      cur_chunk = [0]
                      stag = {}
                      sc_off = [0]
                      pend = []

                      def open_chunk():
                          stag["U"] = spool.tile([128, NS, UCOL], F32,
                                                 tag="ustag", name="ustag")

                      def emit_scatter(ubf, nb, off, h=h):
                          nc.gpsimd.dma_scatter_add(
                              U_half[h][:], ubf[:, :nb, :],
                              scA_sb[h][:, off:off + nb * 8],
                              nb * 128, nb * 128, ROW,
                              single_packet=False, queue_num=_q())

                      def flush_chunk(h=h, sched=sched):
                          ch = cur_chunk[0]
                          nb = sched.chunk_sizes[ch]
                          # full 512B rows (proven 256B-multiple scatter
                          # geometry); pads add zero, zeroed once per
                          # buffer instance and never dirtied after.
                          ubf = spool.tile([128, NS, ROW], BF16,
                                           tag="ubf", name="ubf")
                          if ch < 3:
                              nc.gpsimd.memset(ubf[:, :, UCOL:], 0.0)
                          nc.vector.tensor_copy(ubf[:, :nb, 0:UCOL],
                                                stag["U"][:, :nb, :])
                          # defer the scatter one chunk so its dispatch
                          # never stalls the Pool queue on the cast
                          pend.append((ubf, nb, sc_off[0]))
                          if len(pend) > 1:
                              emit_scatter(*pend.pop(0))
                          sc_off[0] += nb * 8
                          cur_chunk[0] += 1

                      open_chunk()
                      for t in range(sched.ntiles):
                          G = gpool.tile([128, TC, ROW], BF16, tag="G")
                          for (c0s, cc) in _subcols(TC):
                              nc.gpsimd.dma_gather(
                                  G[:, c0s:c0s + cc, :], nfw_table[:],
                                  gA_sb[h][:, t * TC * 8 + c0s * 8:
                                        t * TC * 8 + (c0s + cc) * 8],
                                  cc * 128, cc * 128, ROW,
                                  single_packet=False, queue_num=_q())
                          for (_, c0, k, B, b0, ch) in groups_by_tile.get(t, []):
                              if ch != cur_chunk[0]:
                                  flush_chunk()
                                  open_chunk()
                              bpos = b0 - ch * NS
                              nc.vector.tensor_reduce(
                                  out=stag["U"][:, bpos:bpos + B, :],
                                  in_=_ap(G[:], c0 * ROW,
                                          [[k * ROW, B], [1, UCOL], [ROW, k]]),
                                  axis=AX.X, op=ALU.add)
                      flush_chunk()
                      while pend:
                          emit_scatter(*pend.pop(0))
                    if _stop >= 3:
                        # emitted after this half's pool scope closes, before
                        # the next half's — AR(h) overlaps half h+1 compute
                        nc.gpsimd.collective_compute(
                            "AllReduce", ALU.add,
                            replica_groups=replica_groups,
                            ins=[U_half[h][:]], outs=[Ured_half[h][:]])

            # ---------- edge projection: PE + Scalar only, emitted after
            # passA -- runs fully concurrent with it; ef ready for EA --
            if _stop >= 1:
                with (tc.tile_pool(name="pe", bufs=2) as pe,
                    tc.tile_pool(name="pes", bufs=2) as pes,
                    tc.tile_pool(name="ps2", bufs=4, space="PSUM") as psp2):
                  CHK = 16
                  for a0 in range(0, n_a_edge, CHK):
                      aa = min(CHK, n_a_edge - a0)
                      hc = pe.tile([D, CHK * 128], F32, tag="hc")
                      nc.sync.dma_start(hc[:, :aa * 128], haT[:, a0 * 128:(a0 + aa) * 128])
                      efst = pes.tile([128, CHK, 128], BF16, tag="efst")
                      for i in range(aa):
                          mm = psp2.tile([128, 128], F32, tag="mm")
                          nc.tensor.matmul(mm[:], hc[:, i * 128:(i + 1) * 128], We_sb[:],
                                           start=True, stop=True)
                          nc.scalar.activation(efst[:, i, :], mm[:], ACTF.Identity)
                      nc.sync.dma_start(ef_v[:, a0:a0 + aa, :], efst[:, :aa, :])

            # ---------- y = bias init (needed only by pass B) ----------
            bv = bias_sb[:].rearrange("p (a c) -> p a c", c=H * C)
            for a0 in range(0, n_a_node, 16):
                aa = min(16, n_a_node - a0)
                nc.scalar.dma_start(y_v[:, a0:a0 + aa, :], bv[:, :aa, :])

            # ---------- pass B prologue: index/expl loads (overlap AR) ----------
            if _stop >= 5:
                pb_stack = [tc.tile_pool(name="gB", bufs=6),
                            tc.tile_pool(name="stB", bufs=4),
                            tc.tile_pool(name="idxB", bufs=1)]
                gpoolB, spoolB, ipoolB = [p.__enter__() for p in pb_stack]
                gB_sb = ipoolB.tile([128, schedB.ntiles * TC * 8], I16)
                scB_sb = ipoolB.tile([128, schedB.nbatch * 8], I16)
                rB_sb = ipoolB.tile([128, schedB.nbatch * 8], I16)
                nc.sync.dma_start(gB_sb[:], gB_i[:])
                nc.sync.dma_start(scB_sb[:], scB_i[:])
                nc.sync.dma_start(rB_sb[:], rB_i[:])
                explg = ipoolB.tile([128, schedB.nbatch, 128], BF16)
                for (b0s, bb) in _subcols(schedB.nbatch):
                    nc.gpsimd.dma_gather(
                        explg[:, b0s:b0s + bb, :], expl_table[:],
                        rB_sb[:, b0s * 8:(b0s + bb) * 8],
                        bb * 128, bb * 128, 128,
                        single_packet=False, queue_num=_q())

            # ---------- EA' = (U/D + ef)/D, per half ----------
            if _stop >= 4:
                with tc.tile_pool(name="ea", bufs=2) as eap:
                  for h in range(NH):
                   for a0 in range(0, n_a_h[h], 16):
                      aa = min(16, n_a_h[h] - a0)
                      ga = h * n_a_h[0] + a0
                      uc = eap.tile([128, 16, ROW], BF16, tag="uc")
                      efc = eap.tile([128, 16, 128], BF16, tag="efc")
                      nc.sync.dma_start(uc[:, :aa, :], Ured_v[h][:, a0:a0 + aa, :])
                      nc.sync.dma_start(efc[:, :aa, :], ef_v[:, ga:ga + aa, :])
                      dv = eap.tile([128, 16, H], F32, tag="dv")
                      nc.vector.tensor_copy(dv[:, :aa, :], uc[:, :aa, 128:UCOL])
                      nc.vector.tensor_scalar_add(dv[:, :aa, :], dv[:, :aa, :], 1e-30)
                      inv = eap.tile([128, 16, H], F32, tag="inv")
                      nc.vector.reciprocal(inv[:, :aa, :], dv[:, :aa, :])
                      inv_b = inv[:, :aa, :].unsqueeze(3).broadcast_to([128, aa, H, C])
                      t1 = eap.tile([128, 16, 128], F32, tag="t1")
                      nc.vector.tensor_tensor(
                          out=t1[:, :aa, :].rearrange("p a (h c) -> p a h c", h=H),
                          in0=uc[:, :aa, 0:128].rearrange("p a (h c) -> p a h c", h=H),
                          in1=inv_b, op=ALU.mult)
                      nc.vector.tensor_tensor(out=t1[:, :aa, :], in0=t1[:, :aa, :],
                                              in1=efc[:, :aa, :], op=ALU.add)
                      eab = eap.tile([128, 16, 128], BF16, tag="eab")
                      nc.vector.tensor_tensor(
                          out=eab[:, :aa, :].rearrange("p a (h c) -> p a h c", h=H),
                          in0=t1[:, :aa, :].rearrange("p a (h c) -> p a h c", h=H),
                          in1=inv_b, op=ALU.mult)
                      nc.sync.dma_start(EAp_v[:, ga:ga + aa, :], eab[:, :aa, :])
                  nj = ET - E
                  nc.sync.dma_start(EAp[E:ET, :], zbf[0:nj, 0:128])

            if _dbg:
                if _stop >= 1:
                    nc.sync.dma_start(nfw_dbg[:], nfw_table[:])
                if _stop >= 2:
                    for h in range(NH):
                        nc.sync.dma_start(U_dbg[h * split:h * split + HBs[h], :], U_half[h][:])
                if _stop >= 3:
                    for h in range(NH):
                        nc.sync.dma_start(Ured_dbg[h * split:h * split + HBs[h], :], Ured_half[h][:])
                if _stop >= 4:
                    nc.sync.dma_start(EAp_dbg[:], EAp[:])

            # ---------- pass B ----------
            if _stop >= 5:
                groups_by_tileB = {}
                for g in schedB.groups:
                    groups_by_tileB.setdefault(g[0], []).append(g)
                cur_chunkB = [0]
                stagB = {}
                sc_offB = [0]

                def open_chunkB():
                    stagB["Y"] = spoolB.tile([128, NS, 128], F32, tag="ystag", name="ystag")

                pendB = []

                def emit_scatterB(yst, nb, off):
                    nc.gpsimd.dma_scatter_add(
                        y[:], yst[:, :nb, :],
                        scB_sb[:, off:off + nb * 8],
                        nb * 128, nb * 128, 128,
                        single_packet=False, queue_num=_q())

                def flush_chunkB():
                    ch = cur_chunkB[0]
                    nb = schedB.chunk_sizes[ch]
                    yst = stagB["Y"]
                    nc.vector.tensor_tensor(
                        out=_ap(yst[:], 0, [[128, nb], [32, H], [1, C]]),
                        in0=_ap(yst[:], 0, [[128, nb], [32, H], [1, C]]),
                        in1=_ap(explg[:], ch * NS * 128, [[128, nb], [1, H], [0, C]]),
                        op=ALU.mult)
                    pendB.append((yst, nb, sc_offB[0]))
                    if len(pendB) > 1:
                        emit_scatterB(*pendB.pop(0))
                    sc_offB[0] += nb * 8
                    cur_chunkB[0] += 1

                open_chunkB()
                for t in range(schedB.ntiles):
                    G = gpoolB.tile([128, TC, 128], BF16, tag="G")
                    for (c0s, cc) in _subcols(TC):
                        nc.gpsimd.dma_gather(
                            G[:, c0s:c0s + cc, :], EAp[:],
                            gB_sb[:, t * TC * 8 + c0s * 8:
                                  t * TC * 8 + (c0s + cc) * 8],
                            cc * 128, cc * 128, 128,
                            single_packet=False, queue_num=_q())
                    for (_, c0, k, B, b0, ch) in groups_by_tileB.get(t, []):
                        if ch != cur_chunkB[0]:
                            flush_chunkB()
                            open_chunkB()
                        bpos = b0 - ch * NS
                        nc.vector.tensor_reduce(
                            out=stagB["Y"][:, bpos:bpos + B, :],
                            in_=_ap(G[:], c0 * 128, [[k * 128, B], [1, 128], [128, k]]),
                            axis=AX.X, op=ALU.add)
                flush_chunkB()
                while pendB:
                    emit_scatterB(*pendB.pop(0))
                for p in reversed(pb_stack):
                    p.__exit__(None, None, None)
    nc.compile()
    return nc


def host_inputs(cfg: Cfg, x, ha, W_node, W_edge, attn_l, bias, streams):
    x = np.asarray(x, np.float32)
    ha = np.asarray(ha, np.float32)
    W_node = np.asarray(W_node, np.float32)
    W_edge = np.asarray(W_edge, np.float32)
    attn_flat = np.asarray(attn_l, np.float32).reshape(-1)
    bias = np.asarray(bias, np.float32).reshape(-1)
    attn_rep = np.tile(attn_flat[None, :], (128, 1))
    bias_t = np.tile(bias[None, :], (128, 16))
    ha_pad = np.zeros((cfg.ET_ROWS, cfg.D), np.float32)
    ha_pad[:cfg.E] = ha
    haT = np.ascontiguousarray(ha_pad.T)
    in_maps = []
    for m in range(cfg.NC):
        xs = np.zeros((cfg.NT_ROWS, cfg.D), np.float32)
        xs[:cfg.NSH] = x[m * cfg.NSH:(m + 1) * cfg.NSH]
        st = streams[m]
        in_maps.append({
            "xT": np.ascontiguousarray(xs.T),
            "haT": haT,
            "Wn": W_node, "We": W_edge,
            "attn": attn_rep, "bias_t": bias_t,
            "gB_i": st["gB"], "scB_i": st["scB"], "rB_i": st["rB"],
            **{f"{k}_i": v for k, v in st.items()
               if k.startswith(("gA", "scA"))},
        })
    return in_maps


# ======================== public entry point ========================
_CFG = Cfg()
LAST_RESULTS = None


def _install_axon_ntff_shim():
    import sys, types, ctypes, contextlib
    import concourse.bass_utils as bu
    bu.upload_artifacts = lambda d: str(d)
    try:
        from antenv.axon_hooks import get_axon_ntff_profile_hook  # noqa
        return
    except ImportError:
        pass
    so_path = "/opt/axon/libaxon_pjrt.so"
    try:
        lib = ctypes.CDLL(so_path)
    except OSError:
        return
    if not hasattr(lib, "axon_start_nrt_profile"):
        return
    lib.axon_start_nrt_profile.argtypes = [ctypes.POINTER(ctypes.c_int64),
                                           ctypes.c_size_t]
    lib.axon_start_nrt_profile.restype = ctypes.c_int64
    lib.axon_stop_nrt_profile.argtypes = [ctypes.c_char_p]
    lib.axon_stop_nrt_profile.restype = ctypes.c_int64

    @contextlib.contextmanager
    def _hook(output_dir, device_ids):
        import jax
        jax.devices()
        if device_ids:
            ids = (ctypes.c_int64 * len(device_ids))(*device_ids)
            rc = lib.axon_start_nrt_profile(ids, len(device_ids))
        else:
            rc = lib.axon_start_nrt_profile(None, 0)
        if rc != 0:
            raise RuntimeError(f"axon_start_nrt_profile rc={rc}")
        try:
            yield
        finally:
            n = lib.axon_stop_nrt_profile(str(output_dir).encode())
            print(f"ntff profile: {n} file(s) -> {output_dir}")

    mod = types.ModuleType("antenv.axon_hooks")
    mod.get_axon_ntff_profile_hook = lambda: _hook
    mod.set_axon_ntff_profile_hook = lambda h: None
    sys.modules["antenv.axon_hooks"] = mod


def kernel(**inputs) -> np.ndarray:
    import os
    from concourse.bass_utils import run_bass_kernel_spmd
    cfg = _CFG
    x = np.asarray(inputs["x"], np.float32)
    ha = np.asarray(inputs["hyperedge_attr"], np.float32)
    node_idx = np.asarray(inputs["node_idx"]).astype(np.int64)
    edge_idx = np.asarray(inputs["edge_idx"]).astype(np.int64)
    schedA, schedB, streams = build_plan(node_idx, edge_idx, cfg)
    nc = build_bass(cfg, schedA, schedB, [list(range(cfg.NC))])
    in_maps = host_inputs(cfg, x, ha, inputs["W_node"], inputs["W_edge"],
                          inputs["attn_l"], inputs["bias"], streams)
    trace = os.environ.get("GNN_TRACE", "0") == "1"
    if trace:
        _install_axon_ntff_shim()
    res = run_bass_kernel_spmd(nc, in_maps, list(range(cfg.NC)), trace=trace)
    global LAST_RESULTS
    LAST_RESULTS = res
    out = np.concatenate(
        [np.asarray(res.results[m]["y"])[:cfg.NSH] for m in range(cfg.NC)], axis=0)
    return np.ascontiguousarray(out, dtype=np.float32)


# revision 50
# speedup vs baseline: 1.2977x; 1.0439x over previous
"""Hypergraph conv kernel, v2.

Pipeline (node-sharded, 8 cores):
  phase1: nfw_table[n] = bf16([exp(a_n)*nf_n (128) | exp(a_n) (4) | pad]),
          expl_table[n] = f32 exp(a_n) (for pass B).
  passA:  per edge-run batch: gather nfw rows, ONE reduce -> [U|D] partial,
          cast bf16, scatter-add into U_table[ET,256] (cols 0:132).
  AR:     one bf16 AllReduce of U_table.
  EA:     EAp[e] = bf16((U/D + ef)/D)   (ef projected during passA window)
  passB:  gather EAp rows per incidence, ONE reduce per run group,
          multiply by expl per chunk, scatter-add into y.

SWDGE gathers/scatters run gen_mode=0: descriptor emission on the Pool
Q7 (~3.3ns/row) is the serial spine; transfers overlap it (engine
releases at doorbell; Tile syncs consumers on the DMA sem).
"""
import numpy as np
from dataclasses import dataclass

import concourse.bass as bass
import concourse.mybir as mybir
import concourse.bacc as bacc
import concourse.tile as tile
import bass_rust
from concourse.library_config import mlp as mlp_library
from concourse._compat import get_trn_type, cdiv

F32 = mybir.dt.float32
BF16 = mybir.dt.bfloat16
I16 = mybir.dt.int16
AX = mybir.AxisListType
ALU = mybir.AluOpType
ACTF = mybir.ActivationFunctionType

ROW = 256          # nfw/U table row width (bf16 elems); cols 0:128 nfw, 128:132 expl
UCOL = 132         # useful cols in nfw/U rows


@dataclass
class Cfg:
    N: int = 100000
    E: int = 25000
    D: int = 128
    H: int = 4
    C: int = 32
    NC: int = 8
    TILE_COLS: int = 30
    NSTAGE: int = 21

    @property
    def NSH(self):
        return self.N // self.NC

    @property
    def NT_ROWS(self):
        return cdiv(self.NSH + 1, 128) * 128

    @property
    def ET_ROWS(self):
        return cdiv(self.E + 1, 128) * 128

    @property
    def DUMMY_NODE(self):
        return self.NSH

    @property
    def JUNK_EDGE(self):
        return self.E


def _runs(keys):
    if len(keys) == 0:
        return (np.zeros(0, np.int64),) * 3
    change = np.flatnonzero(np.diff(keys)) + 1
    starts = np.concatenate([[0], change]).astype(np.int64)
    ends = np.concatenate([change, [len(keys)]]).astype(np.int64)
    return starts, ends - starts, keys[starts].astype(np.int64)


@dataclass
class Sched:
    batches: list          # [(k, tile, c0)]
    groups: list           # [(tile, c0, k, B, b0, chunk)]
    ntiles: int
    nchunks: int
    nbatch: int
    chunk_sizes: list


def _mk_schedule(lens_list, cfg: Cfg) -> Sched:
    sorted_lens = [np.sort(np.asarray(l))[::-1] for l in lens_list]
    nbatch_total = max(cdiv(len(l), 128) for l in sorted_lens)
    batches = []
    for b in range(nbatch_total):
        w = 1
        for ls in sorted_lens:
            if b * 128 < len(ls):
                w = max(w, int(ls[b * 128]))
        batches.append(w)
    assert max(batches) <= cfg.TILE_COLS, \
        f"run length {max(batches)} > TILE_COLS"
    placed = []
    t, c = 0, 0
    for k in batches:
        if c + k > cfg.TILE_COLS:
            t += 1
            c = 0
        placed.append((k, t, c))
        c += k
    ntiles = t + 1 if placed else 1
    nbatch = len(placed)
    nchunks = cdiv(nbatch, cfg.NSTAGE)
    chunk_sizes = [min(cfg.NSTAGE, nbatch - i * cfg.NSTAGE) for i in range(nchunks)]
    groups = []
    for bi, (k, t, c0) in enumerate(placed):
        ch = bi // cfg.NSTAGE
        if groups and groups[-1][0] == t and groups[-1][2] == k \
                and groups[-1][5] == ch \
                and groups[-1][1] + groups[-1][2] * groups[-1][3] == c0 \
                and groups[-1][4] + groups[-1][3] == bi:
            t0, c0g, kg, B, b0, chg = groups[-1]
            groups[-1] = (t0, c0g, kg, B + 1, b0, chg)
        else:
            groups.append((t, c0, k, 1, bi, ch))
    return Sched(placed, groups, ntiles, nchunks, nbatch, chunk_sizes)


def _wrap16(flat):
    assert len(flat) % 16 == 0
    b = flat.reshape(-1, 16).T.astype(np.int16)
    return np.tile(b, (8, 1))


SUBMAX = 30


def _subcols(n):
    return [(i, min(SUBMAX, n - i)) for i in range(0, n, SUBMAX)]


def _mk_streams(sched: Sched, starts, lens, gvals, svals, runvals,
                dummy_g, junk_s, dummy_run, cfg: Cfg):
    TC = cfg.TILE_COLS
    g_arr = np.full((sched.ntiles, TC, 128), dummy_g, np.int64)
    s_arr = np.full((sched.nbatch, 128), junk_s, np.int64)
    r_arr = np.full((sched.nbatch, 128), dummy_run, np.int64)
    order = np.argsort(-lens, kind="stable") if len(lens) else np.zeros(0, np.int64)
    for bi, (k, t, c0) in enumerate(sched.batches):
        idxs = order[bi * 128:(bi + 1) * 128]
        nr = len(idxs)
        if nr:
            st = starts[idxs]
            kr = lens[idxs]
            assert kr[0] <= k
            for kk in np.unique(kr):
                sel = np.flatnonzero(kr == kk)
                gm = gvals[st[sel][None, :] + np.arange(kk)[:, None]]
                g_arr[t, c0:c0 + kk, sel] = gm.T
            s_arr[bi, :nr] = svals[idxs]
            r_arr[bi, :nr] = runvals[idxs]
    g_idx = np.concatenate(
        [_wrap16(g_arr[t, c0:c0 + cc].reshape(-1))
         for t in range(sched.ntiles) for (c0, cc) in _subcols(TC)], axis=1)
    sc_blocks = []
    off = 0
    for nb in sched.chunk_sizes:
        sc_blocks.append(_wrap16(s_arr[off:off + nb].reshape(-1)))
        off += nb
    sc_idx = np.concatenate(sc_blocks, axis=1)
    r_idx = np.concatenate(
        [_wrap16(r_arr[b0:b0 + bb].reshape(-1))
         for (b0, bb) in _subcols(sched.nbatch)], axis=1)
    return g_idx, sc_idx, r_idx


def build_plan(node_idx, edge_idx, cfg: Cfg):
    """Pass A is split into two edge-halves (split at ET_ROWS//2, a slot
    boundary) so each half's partial-U AllReduce can overlap the other
    half's compute. Empty-batch scatter slots target row 0 of the half
    table: they add exact zeros (dummy gathers hit the all-zero nfw row),
    so no junk row is needed."""
    node_idx = np.asarray(node_idx).astype(np.int64)
    edge_idx = np.asarray(edge_idx).astype(np.int64)
    bounds = [(0, cfg.ET_ROWS)]
    percore = []
    for m in range(cfg.NC):
        sel = np.flatnonzero(node_idx // cfg.NSH == m)
        nl = node_idx[sel] - m * cfg.NSH
        eg = edge_idx[sel]
        halves = []
        for (lo, hi) in bounds:
            hs = np.flatnonzero((eg >= lo) & (eg < hi))
            sA, lA, vA = _runs(eg[hs])
            halves.append(dict(nl=nl[hs], sA=sA, lA=lA, vA=vA - lo))
        oB = np.argsort(nl, kind="stable")
        nB = nl[oB]
        eB = eg[oB]
        sB, lB, vB = _runs(nB)
        percore.append(dict(halves=halves, eB=eB, sB=sB, lB=lB, vB=vB))
    schedA = [_mk_schedule([c["halves"][h]["lA"] for c in percore], cfg)
              for h in range(len(bounds))]
    schedB = _mk_schedule([c["lB"] for c in percore], cfg)
    streams = []
    for c in percore:
        st = {}
        for h in range(len(c["halves"])):
            ch = c["halves"][h]
            gA, scA, _ = _mk_streams(
                schedA[h], ch["sA"], ch["lA"],
                gvals=ch["nl"], svals=ch["vA"], runvals=ch["vA"],
                dummy_g=cfg.DUMMY_NODE, junk_s=0,
                dummy_run=0, cfg=cfg)
            st[f"gA{h}"] = gA
            st[f"scA{h}"] = scA
        gB, scB, rB = _mk_streams(
            schedB, c["sB"], c["lB"],
            gvals=c["eB"], svals=c["vB"], runvals=c["vB"],
            dummy_g=cfg.JUNK_EDGE, junk_s=cfg.DUMMY_NODE,
            dummy_run=cfg.DUMMY_NODE, cfg=cfg)
        st.update(gB=gB, scB=scB, rB=rB)
        streams.append(st)
    return schedA, schedB, streams


def _ap(t_ap, off, dims):
    base = t_ap
    part = base.ap[0]
    return bass_rust.AP(base.tensor, base.offset + off, [part] + dims)


def build_bass(cfg: Cfg, schedA: Sched, schedB: Sched, replica_groups):
    import os
    _stops = ["init", "phase1", "passA", "coll", "ea", "full"]
    _stop = _stops.index(os.environ.get("GNN_STOP", "full"))
    TC, NS = cfg.TILE_COLS, cfg.NSTAGE
    H, C = cfg.H, cfg.C
    D = cfg.D
    NT, ET = cfg.NT_ROWS, cfg.ET_ROWS
    NSH, E = cfg.NSH, cfg.E
    n_a_node = NT // 128
    n_a_edge = ET // 128

    nc = bacc.Bacc(get_trn_type() or "TRN2", target_bir_lowering=False, debug=False,
                   num_swdge_queues=4)
    _qrr = [0]

    def _q():
        q = _qrr[0] % 4
        _qrr[0] += 1
        return q

    # ---- I/O ----
    xT = nc.dram_tensor("xT", [D, NT], F32, kind="ExternalInput")
    haT = nc.dram_tensor("haT", [D, ET], F32, kind="ExternalInput")
    Wn = nc.dram_tensor("Wn", [D, H * C], F32, kind="ExternalInput")
    We = nc.dram_tensor("We", [D, H * C], F32, kind="ExternalInput")
    attn = nc.dram_tensor("attn", [128, H * C], F32, kind="ExternalInput")
    bias_t = nc.dram_tensor("bias_t", [128, 16 * H * C], F32, kind="ExternalInput")
    NH = len(schedA)
    split = ET if NH == 1 else (ET // 128 // 2) * 128
    HBs = [split, ET - split][:NH]
    n_a_h = [b // 128 for b in HBs]
    gA_i = [nc.dram_tensor(f"gA{h}_i", [128, schedA[h].ntiles * TC * 8], I16,
                           kind="ExternalInput") for h in range(NH)]
    scA_i = [nc.dram_tensor(f"scA{h}_i", [128, schedA[h].nbatch * 8], I16,
                            kind="ExternalInput") for h in range(NH)]
    gB_i = nc.dram_tensor("gB_i", [128, schedB.ntiles * TC * 8], I16, kind="ExternalInput")
    scB_i = nc.dram_tensor("scB_i", [128, schedB.nbatch * 8], I16, kind="ExternalInput")
    rB_i = nc.dram_tensor("rB_i", [128, schedB.nbatch * 8], I16, kind="ExternalInput")
    y = nc.dram_tensor("y", [NT, H * C], F32, kind="ExternalOutput")
    _dbg = os.environ.get("GNN_DEBUG_OUTS", "0") == "1"
    if _dbg:
        nfw_dbg = nc.dram_tensor("nfw_dbg", [NT, ROW], BF16, kind="ExternalOutput")
        U_dbg = nc.dram_tensor("U_dbg", [ET, ROW], BF16, kind="ExternalOutput")
        Ured_dbg = nc.dram_tensor("Ured_dbg", [ET, ROW], BF16, kind="ExternalOutput")
        EAp_dbg = nc.dram_tensor("EAp_dbg", [ET, 128], BF16, kind="ExternalOutput")

    # ---- internal DRAM ----
    nfw_table = nc.dram_tensor("nfw_table", [NT, ROW], BF16)
    expl_table = nc.dram_tensor("expl_table", [NT, 128], BF16)
    ef_table = nc.dram_tensor("ef_table", [ET, 128], BF16)
    U_half = [nc.dram_tensor(f"U_h{h}", [HBs[h], ROW], BF16) for h in range(NH)]
    Ured_half = [nc.dram_tensor(f"Ured_h{h}", [HBs[h], ROW], BF16, addr_space="Shared")
                 for h in range(NH)]
    EAp = nc.dram_tensor("EAp", [ET, 128], BF16)

    nfw_v = nfw_table[:].rearrange("(a p) c -> p a c", p=128)
    expl_v = expl_table[:].rearrange("(a p) c -> p a c", p=128)
    ef_v = ef_table[:].rearrange("(a p) c -> p a c", p=128)
    U_v = [U_half[h][:].rearrange("(a p) c -> p a c", p=128) for h in range(NH)]
    Ured_v = [Ured_half[h][:].rearrange("(a p) c -> p a c", p=128) for h in range(NH)]
    EAp_v = EAp[:].rearrange("(a p) c -> p a c", p=128)
    y_v = y[:].rearrange("(a p) c -> p a c", p=128)

    # SWDGE gathers/scatters run gen_mode=0: the Pool engine emits
    # descriptors (~3.3ns/row) and releases at doorbell; the DMA-completion
    # sem is attached by Tile, so consumer/WAR sync is fully tracked.
    with tile.TileContext(nc) as tc:
        with tc.tile_pool(name="const", bufs=1) as cpool:
            nc.gpsimd.load_library(mlp_library)
            Wn_sb = cpool.tile([D, H * C], F32)
            We_sb = cpool.tile([D, H * C], F32)
            attn_sb = cpool.tile([128, H * C], F32)
            bias_sb = cpool.tile([128, 16 * H * C], F32)
            zbf = cpool.tile([128, 16 * ROW], BF16)
            nc.sync.dma_start(Wn_sb[:], Wn[:])
            nc.sync.dma_start(We_sb[:], We[:])
            nc.sync.dma_start(attn_sb[:], attn[:])
            nc.sync.dma_start(bias_sb[:], bias_t[:])
            nc.gpsimd.memset(zbf[:], 0.0)

            zv = zbf[:].rearrange("p (a c) -> p a c", c=ROW)

            # ---------- phase 1: node projection -> nfw + expl ----------
            if _stop >= 1:
                with (tc.tile_pool(name="p1", bufs=2) as p1,
                    tc.tile_pool(name="p1s", bufs=2) as p1s,
                    tc.tile_pool(name="ps", bufs=4, space="PSUM") as psp):
                  CHK = 16
                  for a0 in range(0, n_a_node, CHK):
                      aa = min(CHK, n_a_node - a0)
                      xc = p1.tile([D, CHK * 128], F32, tag="xc")
                      nc.sync.dma_start(xc[:, :aa * 128], xT[:, a0 * 128:(a0 + aa) * 128])
                      nfst = p1s.tile([128, CHK, 128], F32, tag="nfst")
                      for i in range(aa):
                          mm = psp.tile([128, 128], F32, tag="mm")
                          nc.tensor.matmul(mm[:], xc[:, i * 128:(i + 1) * 128], Wn_sb[:],
                                           start=True, stop=True)
                          nc.vector.tensor_copy(nfst[:, i, :], mm[:])
                      tmp1 = p1s.tile([128, CHK, 128], F32, tag="tmp1")
                      nc.vector.tensor_tensor(
                          out=tmp1[:, :aa, :].rearrange("p a (h c) -> p a h c", h=H),
                          in0=nfst[:, :aa, :].rearrange("p a (h c) -> p a h c", h=H),
                          in1=attn_sb[:].rearrange("p (h c) -> p h c", h=H)
                              .unsqueeze(1).broadcast_to([128, aa, H, C]),
                          op=ALU.mult)
                      praw = p1s.tile([128, CHK, H], F32, tag="praw")
                      nc.vector.tensor_reduce(
                          out=praw[:, :aa, :],
                          in_=tmp1[:, :aa, :].rearrange("p a (h c) -> p a h c", h=H),
                          axis=AX.X, op=ALU.add)
                      estf = p1s.tile([128, CHK, H], F32, tag="estf")
                      nc.scalar.activation(estf[:, :aa, :], praw[:, :aa, :], ACTF.Exp)
                      est = p1s.tile([128, CHK, 128], BF16, tag="est")
                      nc.gpsimd.memset(est[:], 0.0)
                      nc.vector.tensor_copy(est[:, :aa, 0:H], estf[:, :aa, :])
                      nrow = p1s.tile([128, CHK, ROW], BF16, tag="nrow")
                      nc.gpsimd.memset(nrow[:], 0.0)
                      nc.vector.tensor_tensor(
                          out=nrow[:, :aa, 0:128].rearrange("p a (h c) -> p a h c", h=H),
                          in0=nfst[:, :aa, :].rearrange("p a (h c) -> p a h c", h=H),
                          in1=estf[:, :aa, :].unsqueeze(3).broadcast_to([128, aa, H, C]),
                          op=ALU.mult)
                      nc.vector.tensor_copy(nrow[:, :aa, 128:UCOL], estf[:, :aa, :])
                      nc.sync.dma_start(nfw_v[:, a0:a0 + aa, :], nrow[:, :aa, :])
                      nc.sync.dma_start(expl_v[:, a0:a0 + aa, :], est[:, :aa, :])
                  # zero dummy/pad rows (incl. expl cols -> no D contribution)
                  nc.sync.dma_start(nfw_table[NSH:NT, :], zbf[0:NT - NSH, 0:ROW])

            # ---------- U init (emitted late: needed only by scatters) ----
            for h in range(NH):
                for a0 in range(0, n_a_h[h], 16):
                    aa = min(16, n_a_h[h] - a0)
                    nc.scalar.dma_start(U_v[h][:, a0:a0 + aa, :], zv[:, :aa, :])

            # ---------- pass A: two edge-halves, AR per half.
            # Each half gets its own pool scope so the collective is
            # emitted OUTSIDE any pool (inside-pool collectives deadlock
            # Tile's barrier machinery), yet between the halves on the
            # Pool queue so AR(h0) flies during half 1's compute. ----------
            if _stop >= 2:
                  for h in range(NH):
                    with (tc.tile_pool(name=f"gA{h}", bufs=4) as gpool,
                        tc.tile_pool(name=f"stA{h}", bufs=3) as spool,
                        tc.tile_pool(name=f"idxA{h}", bufs=1) as ipool):
                      gA_sb = [None, None]
                      scA_sb = [None, None]
                      gA_sb[h] = ipool.tile([128, schedA[h].ntiles * TC * 8], I16,
                                            name=f"gAsb{h}")
                      scA_sb[h] = ipool.tile([128, schedA[h].nbatch * 8], I16,
                                             name=f"scAsb{h}")
                      nc.sync.dma_start(gA_sb[h][:], gA_i[h][:])
                      nc.sync.dma_start(scA_sb[h][:], scA_i[h][:])
                      sched = schedA[h]
                      groups_by_tile = {}
                      for g in sched.groups:
                          groups_by_tile.setdefault(g[0], []).append(g)

                      cur_chunk = [0]
                      stag = {}
                      sc_off = [0]
                      pend = []

                      def open_chunk():
                          stag["U"] = spool.tile([128, NS, UCOL], F32,
                                                 tag="ustag", name="ustag")

                      def emit_scatter(ubf, nb, off, h=h):
                          nc.gpsimd.dma_scatter_add(
                              U_half[h][:], ubf[:, :nb, :],
                              scA_sb[h][:, off:off + nb * 8],
                              nb * 128, nb * 128, ROW,
                              single_packet=False, queue_num=_q())

                      def flush_chunk(h=h, sched=sched):
                          ch = cur_chunk[0]
                          nb = sched.chunk_sizes[ch]
                          # full 512B rows (proven 256B-multiple scatter
                          # geometry); pads add zero, zeroed once per
                          # buffer instance and never dirtied after.
                          ubf = spool.tile([128, NS, ROW], BF16,
                                           tag="ubf", name="ubf")
                          if ch < 3:
                              nc.gpsimd.memset(ubf[:, :, UCOL:], 0.0)
                          nc.vector.tensor_copy(ubf[:, :nb, 0:UCOL],
                                                stag["U"][:, :nb, :])
                          # defer the scatter one chunk so its dispatch
                          # never stalls the Pool queue on the cast
                          pend.append((ubf, nb, sc_off[0]))
                          if len(pend) > 1:
                              emit_scatter(*pend.pop(0))
                          sc_off[0] += nb * 8
                          cur_chunk[0] += 1

                      open_chunk()
                      for t in range(sched.ntiles):
                          G = gpool.tile([128, TC, ROW], BF16, tag="G")
                          for (c0s, cc) in _subcols(TC):
                              nc.gpsimd.dma_gather(
                                  G[:, c0s:c0s + cc, :], nfw_table[:],
                                  gA_sb[h][:, t * TC * 8 + c0s * 8:
                                        t * TC * 8 + (c0s + cc) * 8],
                                  cc * 128, cc * 128, ROW,
                                  single_packet=False, queue_num=_q())
                          for (_, c0, k, B, b0, ch) in groups_by_tile.get(t, []):
                              if ch != cur_chunk[0]:
                                  flush_chunk()
                                  open_chunk()
                              bpos = b0 - ch * NS
                              nc.vector.tensor_reduce(
                                  out=stag["U"][:, bpos:bpos + B, :],
                                  in_=_ap(G[:], c0 * ROW,
                                          [[k * ROW, B], [1, UCOL], [ROW, k]]),
                                  axis=AX.X, op=ALU.add)
                      flush_chunk()
                      while pend:
                          emit_scatter(*pend.pop(0))
                    if _stop >= 3:
                        # two row-range ARs: EA on the first range can start
                        # while the second range is still reducing
                        mid = (HBs[h] // 256) * 128
                        for (r0, r1) in [(0, mid), (mid, HBs[h])]:
                            nc.gpsimd.collective_compute(
                                "AllReduce", ALU.add,
                                replica_groups=replica_groups,
                                ins=[U_half[h][r0:r1, :]],
                                outs=[Ured_half[h][r0:r1, :]])

            # ---------- edge projection: PE + Scalar only, emitted after
            # passA -- runs fully concurrent with it; ef ready for EA --
            if _stop >= 1:
                with (tc.tile_pool(name="pe", bufs=2) as pe,
                    tc.tile_pool(name="pes", bufs=2) as pes,
                    tc.tile_pool(name="ps2", bufs=4, space="PSUM") as psp2):
                  CHK = 16
                  for a0 in range(0, n_a_edge, CHK):
                      aa = min(CHK, n_a_edge - a0)
                      hc = pe.tile([D, CHK * 128], F32, tag="hc")
                      nc.sync.dma_start(hc[:, :aa * 128], haT[:, a0 * 128:(a0 + aa) * 128])
                      efst = pes.tile([128, CHK, 128], BF16, tag="efst")
                      for i in range(aa):
                          mm = psp2.tile([128, 128], F32, tag="mm")
                          nc.tensor.matmul(mm[:], hc[:, i * 128:(i + 1) * 128], We_sb[:],
                                           start=True, stop=True)
                          nc.scalar.activation(efst[:, i, :], mm[:], ACTF.Identity)
                      nc.sync.dma_start(ef_v[:, a0:a0 + aa, :], efst[:, :aa, :])

            # ---------- y = bias init (needed only by pass B) ----------
            bv = bias_sb[:].rearrange("p (a c) -> p a c", c=H * C)
            for a0 in range(0, n_a_node, 16):
                aa = min(16, n_a_node - a0)
                nc.scalar.dma_start(y_v[:, a0:a0 + aa, :], bv[:, :aa, :])

            # ---------- pass B prologue: index/expl loads (overlap AR) ----------
            if _stop >= 5:
                pb_stack = [tc.tile_pool(name="gB", bufs=6),
                            tc.tile_pool(name="stB", bufs=4),
                            tc.tile_pool(name="idxB", bufs=1)]
                gpoolB, spoolB, ipoolB = [p.__enter__() for p in pb_stack]
                gB_sb = ipoolB.tile([128, schedB.ntiles * TC * 8], I16)
                scB_sb = ipoolB.tile([128, schedB.nbatch * 8], I16)
                rB_sb = ipoolB.tile([128, schedB.nbatch * 8], I16)
                nc.sync.dma_start(gB_sb[:], gB_i[:])
                nc.sync.dma_start(scB_sb[:], scB_i[:])
                nc.sync.dma_start(rB_sb[:], rB_i[:])
                explg = ipoolB.tile([128, schedB.nbatch, 128], BF16)
                for (b0s, bb) in _subcols(schedB.nbatch):
                    nc.gpsimd.dma_gather(
                        explg[:, b0s:b0s + bb, :], expl_table[:],
                        rB_sb[:, b0s * 8:(b0s + bb) * 8],
                        bb * 128, bb * 128, 128,
                        single_packet=False, queue_num=_q())

            # ---------- EA' = (U/D + ef)/D, per half ----------
            if _stop >= 4:
                with tc.tile_pool(name="ea", bufs=2) as eap:
                  for h in range(NH):
                   for a0 in range(0, n_a_h[h], 16):
                      aa = min(16, n_a_h[h] - a0)
                      ga = h * n_a_h[0] + a0
                      uc = eap.tile([128, 16, ROW], BF16, tag="uc")
                      efc = eap.tile([128, 16, 128], BF16, tag="efc")
                      nc.sync.dma_start(uc[:, :aa, :], Ured_v[h][:, a0:a0 + aa, :])
                      nc.sync.dma_start(efc[:, :aa, :], ef_v[:, ga:ga + aa, :])
                      dv = eap.tile([128, 16, H], F32, tag="dv")
                      nc.vector.tensor_copy(dv[:, :aa, :], uc[:, :aa, 128:UCOL])
                      nc.vector.tensor_scalar_add(dv[:, :aa, :], dv[:, :aa, :], 1e-30)
                      inv = eap.tile([128, 16, H], F32, tag="inv")
                      nc.vector.reciprocal(inv[:, :aa, :], dv[:, :aa, :])
                      inv_b = inv[:, :aa, :].unsqueeze(3).broadcast_to([128, aa, H, C])
                      t1 = eap.tile([128, 16, 128], F32, tag="t1")
                      nc.vector.tensor_tensor(
                          out=t1[:, :aa, :].rearrange("p a (h c) -> p a h c", h=H),
                          in0=uc[:, :aa, 0:128].rearrange("p a (h c) -> p a h c", h=H),
                          in1=inv_b, op=ALU.mult)
                      nc.vector.tensor_tensor(out=t1[:, :aa, :], in0=t1[:, :aa, :],
                                              in1=efc[:, :aa, :], op=ALU.add)
                      eab = eap.tile([128, 16, 128], BF16, tag="eab")
                      nc.vector.tensor_tensor(
                          out=eab[:, :aa, :].rearrange("p a (h c) -> p a h c", h=H),
                          in0=t1[:, :aa, :].rearrange("p a (h c) -> p a h c", h=H),
                          in1=inv_b, op=ALU.mult)
                      nc.sync.dma_start(EAp_v[:, ga:ga + aa, :], eab[:, :aa, :])
                  nj = ET - E
                  nc.sync.dma_start(EAp[E:ET, :], zbf[0:nj, 0:128])

            if _dbg:
                if _stop >= 1:
                    nc.sync.dma_start(nfw_dbg[:], nfw_table[:])
                if _stop >= 2:
                    for h in range(NH):
                        nc.sync.dma_start(U_dbg[h * split:h * split + HBs[h], :], U_half[h][:])
                if _stop >= 3:
                    for h in range(NH):
                        nc.sync.dma_start(Ured_dbg[h * split:h * split + HBs[h], :], Ured_half[h][:])
                if _stop >= 4:
                    nc.sync.dma_start(EAp_dbg[:], EAp[:])

            # ---------- pass B ----------
            if _stop >= 5:
                groups_by_tileB = {}
                for g in schedB.groups:
                    groups_by_tileB.setdefault(g[0], []).append(g)
                cur_chunkB = [0]
                stagB = {}
                sc_offB = [0]

                def open_chunkB():
                    stagB["Y"] = spoolB.tile([128, NS, 128], F32, tag="ystag", name="ystag")

                pendB = []

                def emit_scatterB(yst, nb, off):
                    nc.gpsimd.dma_scatter_add(
                        y[:], yst[:, :nb, :],
                        scB_sb[:, off:off + nb * 8],
                        nb * 128, nb * 128, 128,
                        single_packet=False, queue_num=_q())

                def flush_chunkB():
                    ch = cur_chunkB[0]
                    nb = schedB.chunk_sizes[ch]
                    yst = stagB["Y"]
                    nc.vector.tensor_tensor(
                        out=_ap(yst[:], 0, [[128, nb], [32, H], [1, C]]),
                        in0=_ap(yst[:], 0, [[128, nb], [32, H], [1, C]]),
                        in1=_ap(explg[:], ch * NS * 128, [[128, nb], [1, H], [0, C]]),
                        op=ALU.mult)
                    pendB.append((yst, nb, sc_offB[0]))
                    if len(pendB) > 1:
                        emit_scatterB(*pendB.pop(0))
                    sc_offB[0] += nb * 8
                    cur_chunkB[0] += 1

                open_chunkB()
                for t in range(schedB.ntiles):
                    G = gpoolB.tile([128, TC, 128], BF16, tag="G")
                    for (c0s, cc) in _subcols(TC):
                        nc.gpsimd.dma_gather(
                            G[:, c0s:c0s + cc, :], EAp[:],
                            gB_sb[:, t * TC * 8 + c0s * 8:
                                  t * TC * 8 + (c0s + cc) * 8],
                            cc * 128, cc * 128, 128,
                            single_packet=False, queue_num=_q())
                    for (_, c0, k, B, b0, ch) in groups_by_tileB.get(t, []):
                        if ch != cur_chunkB[0]:
                            flush_chunkB()
                            open_chunkB()
                        bpos = b0 - ch * NS
                        nc.vector.tensor_reduce(
                            out=stagB["Y"][:, bpos:bpos + B, :],
                            in_=_ap(G[:], c0 * 128, [[k * 128, B], [1, 128], [128, k]]),
                            axis=AX.X, op=ALU.add)
                flush_chunkB()
                while pendB:
                    emit_scatterB(*pendB.pop(0))
                for p in reversed(pb_stack):
                    p.__exit__(None, None, None)
    nc.compile()
    return nc


def host_inputs(cfg: Cfg, x, ha, W_node, W_edge, attn_l, bias, streams):
    x = np.asarray(x, np.float32)
    ha = np.asarray(ha, np.float32)
    W_node = np.asarray(W_node, np.float32)
    W_edge = np.asarray(W_edge, np.float32)
    attn_flat = np.asarray(attn_l, np.float32).reshape(-1)
    bias = np.asarray(bias, np.float32).reshape(-1)
    attn_rep = np.tile(attn_flat[None, :], (128, 1))
    bias_t = np.tile(bias[None, :], (128, 16))
    ha_pad = np.zeros((cfg.ET_ROWS, cfg.D), np.float32)
    ha_pad[:cfg.E] = ha
    haT = np.ascontiguousarray(ha_pad.T)
    in_maps = []
    for m in range(cfg.NC):
        xs = np.zeros((cfg.NT_ROWS, cfg.D), np.float32)
        xs[:cfg.NSH] = x[m * cfg.NSH:(m + 1) * cfg.NSH]
        st = streams[m]
        in_maps.append({
            "xT": np.ascontiguousarray(xs.T),
            "haT": haT,
            "Wn": W_node, "We": W_edge,
            "attn": attn_rep, "bias_t": bias_t,
            "gB_i": st["gB"], "scB_i": st["scB"], "rB_i": st["rB"],
            **{f"{k}_i": v for k, v in st.items()
               if k.startswith(("gA", "scA"))},
        })
    return in_maps


# ======================== public entry point ========================
_CFG = Cfg()
LAST_RESULTS = None


def _install_axon_ntff_shim():
    import sys, types, ctypes, contextlib
    import concourse.bass_utils as bu
    bu.upload_artifacts = lambda d: str(d)
    try:
        from antenv.axon_hooks import get_axon_ntff_profile_hook  # noqa
        return
    except ImportError:
        pass
    so_path = "/opt/axon/libaxon_pjrt.so"
    try:
        lib = ctypes.CDLL(so_path)
    except OSError:
        return
    if not hasattr(lib, "axon_start_nrt_profile"):
        return
    lib.axon_start_nrt_profile.argtypes = [ctypes.POINTER(ctypes.c_int64),
                                           ctypes.c_size_t]
    lib.axon_start_nrt_profile.restype = ctypes.c_int64
    lib.axon_stop_nrt_profile.argtypes = [ctypes.c_char_p]
    lib.axon_stop_nrt_profile.restype = ctypes.c_int64

    @contextlib.contextmanager
    def _hook(output_dir, device_ids):
        import jax
        jax.devices()
        if device_ids:
            ids = (ctypes.c_int64 * len(device_ids))(*device_ids)
            rc = lib.axon_start_nrt_profile(ids, len(device_ids))
        else:
            rc = lib.axon_start_nrt_profile(None, 0)
        if rc != 0:
            raise RuntimeError(f"axon_start_nrt_profile rc={rc}")
        try:
            yield
        finally:
            n = lib.axon_stop_nrt_profile(str(output_dir).encode())
            print(f"ntff profile: {n} file(s) -> {output_dir}")

    mod = types.ModuleType("antenv.axon_hooks")
    mod.get_axon_ntff_profile_hook = lambda: _hook
    mod.set_axon_ntff_profile_hook = lambda h: None
    sys.modules["antenv.axon_hooks"] = mod


def kernel(**inputs) -> np.ndarray:
    import os
    from concourse.bass_utils import run_bass_kernel_spmd
    cfg = _CFG
    x = np.asarray(inputs["x"], np.float32)
    ha = np.asarray(inputs["hyperedge_attr"], np.float32)
    node_idx = np.asarray(inputs["node_idx"]).astype(np.int64)
    edge_idx = np.asarray(inputs["edge_idx"]).astype(np.int64)
    schedA, schedB, streams = build_plan(node_idx, edge_idx, cfg)
    nc = build_bass(cfg, schedA, schedB, [list(range(cfg.NC))])
    in_maps = host_inputs(cfg, x, ha, inputs["W_node"], inputs["W_edge"],
                          inputs["attn_l"], inputs["bias"], streams)
    trace = os.environ.get("GNN_TRACE", "0") == "1"
    if trace:
        _install_axon_ntff_shim()
    res = run_bass_kernel_spmd(nc, in_maps, list(range(cfg.NC)), trace=trace)
    global LAST_RESULTS
    LAST_RESULTS = res
    out = np.concatenate(
        [np.asarray(res.results[m]["y"])[:cfg.NSH] for m in range(cfg.NC)], axis=0)
    return np.ascontiguousarray(out, dtype=np.float32)
